# revision 1
# baseline (speedup 1.0000x reference)
"""Trainium2 Bass kernel for EnhancedGATModel (3-layer GATv2, N=50000, E=800000).

Strategy (8 NeuronCores, graph-partitioned by destination node):
- Host: append self-loops, sort edges by dst, partition dst nodes 6250/core,
  bucket edges per 128-dst block, split each block's edges by src half
  (int16 gather indices), pad to 128-edge tiles uniformly across cores.
- Device (single SPMD NEFF on 8 cores):
  * per-layer node tables xl = h@Wl (own shard) + AllGather -> full table
  * per block: dma_gather of xl[src] rows, segment softmax + weighted
    segment-sum via one-hot matmuls accumulating in PSUM (denominator as an
    appended ones-column; no max-subtraction -- logits are tiny)
  * BN/relu/residual per node block; final log_softmax via Softplus.
"""
import sys
import numpy as np

sys.path.insert(0, "/opt/trn_rl_repo")

import concourse.bass as bass
import concourse.mybir as mybir
import concourse.tile as tile
from concourse import bacc
from concourse.bass_utils import run_bass_kernel_spmd
from concourse.masks import make_identity

F32 = mybir.dt.float32
I16 = mybir.dt.int16
AF = mybir.ActivationFunctionType
ALU = mybir.AluOpType

NCORES = 8
BLOCK = 128
D_IN, HID, HEADS, OUT = 128, 64, 4, 2
HC = HEADS * HID  # 256
NEG_SLOPE = 0.2
BN_EPS = 1e-5
PRELU_VIA_DVE = False  # CoreSim lacks Prelu; flip for simulation
GATHER_FROM_OWN = False  # debug: bypass Shared table (single-core only)
NUM_GATHERS = 3  # debug: 1=xl gathers only(no xr), 2=+xr, 3=all (normal)
SIMPLE_A = False  # debug: replace phase A with dummy table writes
CHAIN_DEPTH = 9  # debug: per-tile chain depth
MAX_PHASE = 7  # debug bisection: 1=A 2=+AG0+B 3=+C+AG1 4=+D 5=+E+AG2 6=+F 7=+G
EDGE_LEVEL = 4  # 1=gathers 2=+tile chain(no mm) 3=+scatter mm 4=full post


# ---------------------------------------------------------------- host prep
def preprocess(edge_index, N):
    """Returns per-core gather index / dst-local arrays and the tile schedule.

    schedule: list of (block, 'lo'|'hi', ntiles) in tile order; uniform
    across cores. Edge k of a (core,block,half) group lands at partition
    k%128 of tile k//128; pads use src-index 0 (finite reads) and
    dst_local=200 (masked out of the one-hot).
    """
    NPC = N // NCORES
    NBLK = (NPC + BLOCK - 1) // BLOCK
    HALF = N // 2
    src = np.concatenate([edge_index[0], np.arange(N)]).astype(np.int64)
    dst = np.concatenate([edge_index[1], np.arange(N)]).astype(np.int64)
    order = np.argsort(dst, kind="stable")
    src, dst = src[order], dst[order]
    core_of = dst // NPC
    # group edges by (core, block, half)
    groups = {}
    for c in range(NCORES):
        m = core_of == c
        sc, dc = src[m], dst[m]
        loc = dc - c * NPC
        blk = loc // BLOCK
        lo = sc < HALF
        for b in range(NBLK):
            mb = blk == b
            groups[(c, b, 0)] = (sc[mb & lo], loc[mb & lo] % BLOCK)
            groups[(c, b, 1)] = (sc[mb & ~lo] - HALF, loc[mb & ~lo] % BLOCK)
    schedule = []
    for b in range(NBLK):
        for h, nm in ((0, "lo"), (1, "hi")):
            mx = max(len(groups[(c, b, h)][0]) for c in range(NCORES))
            T = (mx + 127) // 128
            if T > 0:
                schedule.append((b, nm, T))
    TT = sum(T for _, _, T in schedule)
    idx_xl = np.zeros((NCORES, 128, 8 * TT), np.int16)
    idx_xr = np.zeros((NCORES, 128, 8 * TT), np.int16)
    dstl = np.full((NCORES, 128, TT), 200.0, np.float32)
    t0 = 0
    for b, nm, T in schedule:
        h = 0 if nm == "lo" else 1
        for c in range(NCORES):
            s, dl = groups[(c, b, h)]
            ne = len(s)
            pad = T * 128 - ne
            sp = np.concatenate([s, np.zeros(pad, np.int64)]).astype(np.int64)
            dlp = np.concatenate([dl, np.full(pad, 200)]).astype(np.int64)
            # xl gather index list (edge order), wrapped [16, 8T] col-major
            wrap = sp.reshape(8 * T, 16).T.astype(np.int16)
            idx_xl[c, :, 8 * t0:8 * (t0 + T)] = np.tile(wrap, (8, 1))
            # xr index: own-node row = b*128 + clamped dst local
            xr = b * BLOCK + np.minimum(dlp, BLOCK - 1)
            xr = np.minimum(xr, N // NCORES - 1)
            wrap2 = xr.reshape(8 * T, 16).T.astype(np.int16)
            idx_xr[c, :, 8 * t0:8 * (t0 + T)] = np.tile(wrap2, (8, 1))
            dstl[c, :, t0:t0 + T] = dlp.reshape(T, 128).T.astype(np.float32)
        t0 += T
    return idx_xl, idx_xr, dstl, schedule, NBLK


def pack_consts(ip, N):
    """Pack all small constants/weights into one [128, CW] f32 tensor.
    Returns (array, dict of name -> (row0, rows, col0, cols))."""
    cols = {}
    parts = []
    c0 = [0]

    def add(name, arr, rows=128):
        a = np.zeros((128, arr.shape[1]), np.float32)
        a[:arr.shape[0]] = arr
        cols[name] = (arr.shape[0], c0[0], arr.shape[1])
        parts.append(a)
        c0[0] += arr.shape[1]

    iota = np.broadcast_to(np.arange(128, dtype=np.float32), (128, 128))
    add("iota", np.ascontiguousarray(iota))
    add("iotaC", np.arange(128, dtype=np.float32)[:, None])
    bcast = lambda v: np.broadcast_to(np.asarray(v, np.float32)[None, :], (128, len(np.asarray(v)))).copy()
    att0 = np.asarray(ip["att0"], np.float32).reshape(-1)   # [4*64]
    att1 = np.asarray(ip["att1"], np.float32).reshape(-1)
    att2 = np.asarray(ip["att2"], np.float32).reshape(-1)   # [2]
    add("attB0", bcast(att0))
    add("attB1", bcast(att1))
    add("attB2", bcast(att2))
    g, bt = np.asarray(ip["bn_gamma"]), np.asarray(ip["bn_beta"])
    mu, var = np.asarray(ip["bn_mean"]), np.asarray(ip["bn_var"])
    for l in range(2):
        a = g[l] / np.sqrt(var[l] + BN_EPS)
        bias_l = np.asarray(ip[f"bias{l}"], np.float32)
        b = bt[l] - mu[l] * a + a * bias_l
        add(f"aB{l}", bcast(a))
        add(f"bB{l}", bcast(b))
    add("bias2B", bcast(np.asarray(ip["bias2"], np.float32)))
    add("W_in", np.asarray(ip["W_in"], np.float32))          # [128, 64]
    add("b_in", np.asarray(ip["b_in"], np.float32).reshape(-1, 1))
    add("Wl0", np.asarray(ip["Wl0"], np.float32))            # [64, 256]
    add("Wr0", np.asarray(ip["Wr0"], np.float32))
    Wl1, Wr1 = np.asarray(ip["Wl1"], np.float32), np.asarray(ip["Wr1"], np.float32)
    add("Wl1k0", Wl1[:128]); add("Wl1k1", Wl1[128:])
    add("Wr1k0", Wr1[:128]); add("Wr1k1", Wr1[128:])
    Wl2, Wr2 = np.asarray(ip["Wl2"], np.float32), np.asarray(ip["Wr2"], np.float32)
    add("Wl2k0", Wl2[:128]); add("Wl2k1", Wl2[128:])
    add("Wr2k0", Wr2[:128]); add("Wr2k1", Wr2[128:])
    return np.concatenate(parts, axis=1), cols



GMAX = 8  # dma_gather hardware limit: <=1024 indices per op


def _gather(nc, out_tile, in_ap, idx_tile, tstart, T, elem):
    """Chunked dma_gather: out_tile[:, k:k+Tc, :] = table[idx[tile k..]]."""
    k = 0
    while k < T:
        Tc = min(GMAX, T - k)
        nc.gpsimd.dma_gather(
            out_ap=out_tile[:, k:k + Tc, :], in_ap=in_ap,
            idxs_ap=idx_tile[:, 8 * (tstart + k):8 * (tstart + k + Tc)],
            num_idxs=128 * Tc, num_idxs_reg=128 * Tc, elem_size=elem)
        k += Tc


def _chunks(NPC):
    out = []
    st = 0
    while st < NPC:
        out.append((st, min(128, NPC - st)))
        st += 128
    return out


def build(N, schedule, NBLK, TT, CW):
    NPC = N // NCORES
    HALF = N // 2
    nc = bacc.Bacc("TRN2", target_bir_lowering=False, debug=False)

    xT = nc.dram_tensor("xT", [D_IN, NPC], F32, kind="ExternalInput")
    idx_xl = nc.dram_tensor("idx_xl", [128, 8 * TT], I16, kind="ExternalInput")
    idx_xr = nc.dram_tensor("idx_xr", [128, 8 * TT], I16, kind="ExternalInput")
    dstl = nc.dram_tensor("dstl", [128, TT], F32, kind="ExternalInput")
    consts = nc.dram_tensor("consts", [128, CW], F32, kind="ExternalInput")
    out = nc.dram_tensor("out", [NPC, OUT], F32, kind="ExternalOutput")

    xl0_own = nc.dram_tensor("xl0_own", [NPC, HC], F32)
    xl0_full = nc.dram_tensor("xl0_full", [N, HC], F32, addr_space="Shared")
    xr0 = nc.dram_tensor("xr0", [NPC, HC], F32)
    h1_own = nc.dram_tensor("h1_own", [NPC, HC], F32)
    h1T = nc.dram_tensor("h1T", [HC, NPC], F32)
    xl1_own = nc.dram_tensor("xl1_own", [NPC, HC], F32)
    xl1_full = nc.dram_tensor("xl1_full", [N, HC], F32, addr_space="Shared")
    xr1 = nc.dram_tensor("xr1", [NPC, HC], F32)
    h2T = nc.dram_tensor("h2T", [HC, NPC], F32)
    xl2p_own = nc.dram_tensor("xl2p_own", [NPC, 64], F32)
    xl2p_full = nc.dram_tensor("xl2p_full", [N, 64], F32, addr_space="Shared")
    xr2p = nc.dram_tensor("xr2p", [NPC, 64], F32)

    chunks = _chunks(NPC)
    # block tile layout: per block b, list of (tile_start, 'lo'|'hi', T)
    blk_tiles = {b: [] for b in range(NBLK)}
    t0 = 0
    for b, nm, T in schedule:
        blk_tiles[b].append((t0, nm, T))
        t0 += T

    rg = [list(range(NCORES))]

    with tile.TileContext(nc) as tc:
        import contextlib
        with contextlib.ExitStack() as ctx:
            cst = ctx.enter_context(tc.tile_pool(name="cst", bufs=1))
            sb = ctx.enter_context(tc.tile_pool(name="sb", bufs=3))
            gat = ctx.enter_context(tc.tile_pool(name="gat", bufs=2))
            ps = ctx.enter_context(tc.tile_pool(name="ps", bufs=2, space="PSUM"))
            psa = ctx.enter_context(tc.tile_pool(name="psa", bufs=2, space="PSUM"))

            C = cst.tile([128, CW], F32)
            nc.sync.dma_start(C[:], consts[:])
            ident = cst.tile([128, 128], F32)
            nc.vector.tensor_scalar(out=ident[:], in0=C[0:128, COLS["iota"][1]:COLS["iota"][1] + 128],
                                    scalar1=C[0:128, COLS["iotaC"][1]:COLS["iotaC"][1] + 1],
                                    scalar2=None, op0=ALU.is_equal)
            ixl_t = cst.tile([128, 8 * TT], I16)
            nc.sync.dma_start(ixl_t[:], idx_xl[:])
            ixr_t = cst.tile([128, 8 * TT], I16)
            nc.sync.dma_start(ixr_t[:], idx_xr[:])
            dstl_t = cst.tile([128, TT], F32)
            nc.sync.dma_start(dstl_t[:], dstl[:])
            o_all = cst.tile([128, 2 * NBLK], F32)

            def cs(name):
                r, c0i, w = COLS[name]
                return C[0:r, c0i:c0i + w]

            # ---------------- phase A: L0 node prep ----------------
            if SIMPLE_A:
                for st, sz in chunks:
                    cpz = sb.tile([128, HC], F32, tag="cpA")
                    nc.vector.tensor_copy(cpz[:], C[:, 0:HC])
                    nc.sync.dma_start(xl0_own[st:st + sz, :], cpz[:sz, :])
                    cpz2 = sb.tile([128, HC], F32, tag="cpA")
                    nc.vector.tensor_copy(cpz2[:], C[:, 0:HC])
                    nc.sync.dma_start(xr0[st:st + sz, :], cpz2[:sz, :])
            for st, sz in (chunks if not SIMPLE_A else []):
                xTc = sb.tile([D_IN, 128], F32, tag="xTc")
                nc.sync.dma_start(xTc[:, :sz], xT[:, st:st + sz])
                p1 = psa.tile([64, 128], F32, tag="prep", space="PSUM")
                nc.tensor.matmul(p1[:, :sz], lhsT=cs("W_in"), rhs=xTc[:, :sz],
                                 start=True, stop=True)
                h0T = sb.tile([64, 128], F32, tag="h0T")
                nc.scalar.activation(h0T[:, :sz], p1[:, :sz], AF.Relu,
                                     bias=cs("b_in"))
                for W, tab in (("Wl0", xl0_own), ("Wr0", xr0)):
                    p2 = psa.tile([128, HC], F32, tag="prep", space="PSUM")
                    nc.tensor.matmul(p2[:sz, :], lhsT=h0T[:, :sz], rhs=cs(W),
                                     start=True, stop=True)
                    cp = sb.tile([128, HC], F32, tag="cpA")
                    nc.scalar.copy(cp[:sz, :], p2[:sz, :])
                    nc.sync.dma_start(tab[st:st + sz, :], cp[:sz, :])

            if MAX_PHASE >= 2 and not GATHER_FROM_OWN:
                nc.gpsimd.collective_compute(
                    "AllGather", ALU.bypass, ins=[xl0_own[:]], outs=[xl0_full[:]],
                    replica_groups=rg)

            # ---------------- edge pass for layers 0/1 ----------------
            def edge_pass(lidx, xl_full, xr_tab, attB, aB, bB, hT_out, h_own_out,
                          residual):
                for b in range(NBLK):
                    st = b * BLOCK
                    nreal = min(BLOCK, NPC - st)
                    tl = blk_tiles[b]
                    T_all = sum(T for _, _, T in tl)
                    tglob0 = tl[0][0]
                    # gathers
                    gts = []
                    for (tg, nm, T) in tl:
                        g = gat.tile([128, T, HC], F32, tag=f"gxl_{nm}")
                        src_ap = xl_full[0:HALF, :] if nm == "lo" else xl_full[HALF:N, :]
                        _gather(nc, g, src_ap, ixl_t, tg, T, HC)
                        gts.append((g, T))
                    xr_g = gat.tile([128, T_all, HC], F32, tag="gxr")
                    if NUM_GATHERS >= 2:
                        _gather(nc, xr_g, xr_tab[:], ixr_t, tglob0, T_all, HC)
                    if residual is not None:
                        hres = sb.tile([128, HC], F32, tag="hres")
                        nc.sync.dma_start(hres[:nreal, :], residual[st:st + nreal, :])
                    if EDGE_LEVEL < 2:
                        cp0 = sb.tile([128, HC], F32, tag="cpA")
                        nc.vector.tensor_copy(cp0[:], gts[0][0][:, 0, :])
                        nc.sync.dma_start(h1_own[st:st + nreal, :] if h_own_out is not None else h1T[0:128, st:st+nreal], cp0[:nreal, :] if h_own_out is not None else cp0[:, :nreal])
                        continue
                    acc = ps.tile([128, HEADS, HID + 1], F32, tag="acc", space="PSUM")
                    tloc = 0
                    for gi, (g, T) in enumerate(gts):
                        for t in range(T):
                            gt = tglob0 + tloc
                            xl_s = g[:, t, :]
                            u = sb.tile([128, HC], F32, tag="u")
                            nc.vector.tensor_tensor(out=u[:], in0=xl_s, in1=(xr_g[:, tloc, :] if NUM_GATHERS >= 2 else xl_s), op=ALU.add)
                            if CHAIN_DEPTH == 1:
                                if tloc == 0:
                                    nc.sync.dma_start(h1T[0:128, st:st + nreal], u[:, :nreal])
                                tloc += 1
                                continue
                            v = sb.tile([128, HC], F32, tag="v")
                            if PRELU_VIA_DVE:
                                nc.vector.tensor_scalar(out=v[:], in0=u[:], scalar1=NEG_SLOPE,
                                                        scalar2=None, op0=ALU.mult)
                                nc.vector.tensor_tensor(out=v[:], in0=v[:], in1=u[:], op=ALU.max)
                            else:
                                nc.scalar.activation(v[:], u[:], AF.Prelu, alpha=NEG_SLOPE)
                            if CHAIN_DEPTH == 2:
                                if tloc == 0:
                                    nc.sync.dma_start(h1T[0:128, st:st + nreal], v[:, :nreal])
                                tloc += 1
                                continue
                            p = sb.tile([128, HEADS, HID], F32, tag="p")
                            nc.vector.tensor_tensor(out=p[:].rearrange("a h c -> a (h c)"),
                                                    in0=v[:], in1=attB, op=ALU.mult)
                            lg = sb.tile([128, HEADS], F32, tag="lg")
                            nc.vector.tensor_reduce(out=lg[:], in_=p[:], axis=mybir.AxisListType.X,
                                                    op=ALU.add)
                            if CHAIN_DEPTH == 3:
                                if tloc == 0:
                                    nc.sync.dma_start(h1T[0:4, st:st + nreal], lg[:, :nreal].rearrange("a b -> b a"))
                                tloc += 1
                                continue
                            ex = sb.tile([128, HEADS], F32, tag="ex")
                            nc.scalar.activation(ex[:], lg[:], AF.Exp)
                            if CHAIN_DEPTH == 4:
                                if tloc == 0:
                                    nc.sync.dma_start(h1T[0:4, st:st + nreal], ex[:, :nreal].rearrange("a b -> b a"))
                                tloc += 1
                                continue
                            oh = sb.tile([128, 128], F32, tag="oh")
                            nc.vector.tensor_scalar(out=oh[:], in0=cs("iota"),
                                                    scalar1=dstl_t[:, gt:gt + 1], scalar2=None,
                                                    op0=ALU.is_equal)
                            if CHAIN_DEPTH == 5:
                                if tloc == 0:
                                    nc.sync.dma_start(h1T[0:128, st:st + nreal], oh[:, :nreal])
                                tloc += 1
                                continue
                            rhs = sb.tile([128, HEADS, HID + 1], F32, tag="rhs")
                            nc.vector.tensor_tensor(
                                out=rhs[:, :, 0:HID],
                                in0=xl_s.rearrange("a (h c) -> a h c", h=HEADS),
                                in1=ex[:, :, None].to_broadcast([128, HEADS, HID]),
                                op=ALU.mult)
                            if CHAIN_DEPTH == 6:
                                if tloc == 0:
                                    nc.sync.dma_start(h1T[0:128, st:st + nreal], rhs[:].rearrange("a h c -> a (h c)")[:, 0:128][:, :nreal])
                                tloc += 1
                                continue
                            nc.scalar.copy(rhs[:, :, HID:HID + 1], ex[:, :, None])
                            if CHAIN_DEPTH == 7:
                                if tloc == 0:
                                    nc.sync.dma_start(h1T[0:128, st:st + nreal], rhs[:].rearrange("a h c -> a (h c)")[:, 0:128][:, :nreal])
                                tloc += 1
                                continue
                            if EDGE_LEVEL >= 3:
                                nc.tensor.matmul(
                                    acc[:].rearrange("a h c -> a (h c)"),
                                    lhsT=oh[:], rhs=rhs[:].rearrange("a h c -> a (h c)"),
                                    start=(tloc == 0), stop=(tloc == T_all - 1))
                            tloc += 1
                    # block post
                    if CHAIN_DEPTH < 8:
                        continue
                    if EDGE_LEVEL < 4:
                        cp1 = sb.tile([128, HC], F32, tag="cpA")
                        src_post = acc[:].rearrange("a h c -> a (h c)")[:, 0:HC] if EDGE_LEVEL >= 3 else rhs[:].rearrange("a h c -> a (h c)")[:, 0:HC]
                        nc.scalar.copy(cp1[:], src_post)
                        if h_own_out is not None:
                            nc.sync.dma_start(h_own_out[st:st + nreal, :], cp1[:nreal, :])
                        for half in range(2):
                            nc.sync.dma_start(hT_out[half * 128:(half + 1) * 128, st:st + nreal], cp1[:, :nreal])
                        continue
                    rc = sb.tile([128, HEADS], F32, tag="rc")
                    nc.vector.reciprocal(rc[:], acc[:, :, HID:HID + 1])
                    go = sb.tile([128, HC], F32, tag="go")
                    nc.vector.tensor_tensor(
                        out=go[:].rearrange("a (h c) -> a h c", h=HEADS),
                        in0=acc[:, :, 0:HID],
                        in1=rc[:, :, None].to_broadcast([128, HEADS, HID]),
                        op=ALU.mult)
                    t1 = sb.tile([128, HC], F32, tag="t1")
                    nc.vector.tensor_tensor(out=t1[:], in0=go[:], in1=aB, op=ALU.mult)
                    t2 = sb.tile([128, HC], F32, tag="t2")
                    nc.vector.tensor_tensor(out=t2[:], in0=t1[:], in1=bB, op=ALU.add)
                    h = sb.tile([128, HC], F32, tag="h")
                    nc.scalar.activation(h[:], t2[:], AF.Relu)
                    if residual is not None:
                        h2 = sb.tile([128, HC], F32, tag="h2")
                        nc.vector.tensor_tensor(out=h2[:nreal, :], in0=h[:nreal, :],
                                                in1=hres[:nreal, :], op=ALU.add)
                        h = h2
                    if h_own_out is not None:
                        nc.sync.dma_start(h_own_out[st:st + nreal, :], h[:nreal, :])
                    for half in range(2):
                        tp = ps.tile([128, 128], F32, tag="tp", space="PSUM")
                        nc.tensor.transpose(tp[:], h[:, half * 128:(half + 1) * 128], ident[:])
                        tcp = sb.tile([128, 128], F32, tag="tcp")
                        nc.scalar.copy(tcp[:], tp[:])
                        nc.sync.dma_start(hT_out[half * 128:(half + 1) * 128, st:st + nreal],
                                          tcp[:, :nreal])

            if MAX_PHASE >= 2.5:
                edge_pass(0, xl0_own if GATHER_FROM_OWN else xl0_full, xr0, cs("attB0"), cs("aB0"), cs("bB0"),
                          h1T, h1_own, None)

            # ---------------- phase C: L1 node prep ----------------
            for st, sz in (chunks if MAX_PHASE >= 3 else []):
                ht0 = sb.tile([128, 128], F32, tag="ht0")
                nc.sync.dma_start(ht0[:, :sz], h1T[0:128, st:st + sz])
                ht1 = sb.tile([128, 128], F32, tag="ht1")
                nc.sync.dma_start(ht1[:, :sz], h1T[128:256, st:st + sz])
                for Wk0, Wk1, tab in (("Wl1k0", "Wl1k1", xl1_own), ("Wr1k0", "Wr1k1", xr1)):
                    p2 = psa.tile([128, HC], F32, tag="prep", space="PSUM")
                    nc.tensor.matmul(p2[:sz, :], lhsT=ht0[:, :sz], rhs=cs(Wk0), start=True, stop=False)
                    nc.tensor.matmul(p2[:sz, :], lhsT=ht1[:, :sz], rhs=cs(Wk1), start=False, stop=True)
                    cp = sb.tile([128, HC], F32, tag="cpA")
                    nc.scalar.copy(cp[:sz, :], p2[:sz, :])
                    nc.sync.dma_start(tab[st:st + sz, :], cp[:sz, :])

            if MAX_PHASE >= 3:
                nc.gpsimd.collective_compute(
                    "AllGather", ALU.bypass, ins=[xl1_own[:]], outs=[xl1_full[:]],
                    replica_groups=rg)

            if MAX_PHASE >= 4:
                edge_pass(1, xl1_full, xr1, cs("attB1"), cs("aB1"), cs("bB1"),
                          h2T, None, h1_own)

            # ---------------- phase E: L2 node prep ----------------
            for st, sz in (chunks if MAX_PHASE >= 5 else []):
                ht0 = sb.tile([128, 128], F32, tag="ht0")
                nc.sync.dma_start(ht0[:, :sz], h2T[0:128, st:st + sz])
                ht1 = sb.tile([128, 128], F32, tag="ht1")
                nc.sync.dma_start(ht1[:, :sz], h2T[128:256, st:st + sz])
                for Wk0, Wk1, tab in (("Wl2k0", "Wl2k1", xl2p_own), ("Wr2k0", "Wr2k1", xr2p)):
                    p2 = psa.tile([128, OUT], F32, tag="prep", space="PSUM")
                    nc.tensor.matmul(p2[:sz, :], lhsT=ht0[:, :sz], rhs=cs(Wk0), start=True, stop=False)
                    nc.tensor.matmul(p2[:sz, :], lhsT=ht1[:, :sz], rhs=cs(Wk1), start=False, stop=True)
                    cp = sb.tile([128, 64], F32, tag="cpE")
                    nc.vector.memset(cp[:], 0.0)
                    nc.scalar.copy(cp[:sz, 0:OUT], p2[:sz, :])
                    nc.sync.dma_start(tab[st:st + sz, :], cp[:sz, :])

            if MAX_PHASE >= 5:
                nc.gpsimd.collective_compute(
                    "AllGather", ALU.bypass, ins=[xl2p_own[:]], outs=[xl2p_full[:]],
                    replica_groups=rg)

            # ---------------- phase F: L2 edge pass ----------------
            att2 = cs("attB2")
            for b in (range(NBLK) if MAX_PHASE >= 6 else []):
                st = b * BLOCK
                nreal = min(BLOCK, NPC - st)
                tl = blk_tiles[b]
                T_all = sum(T for _, _, T in tl)
                tglob0 = tl[0][0]
                gts = []
                for (tg, nm, T) in tl:
                    g = gat.tile([128, T, 64], F32, tag=f"gxl_{nm}")
                    src_ap = xl2p_full[0:HALF, :] if nm == "lo" else xl2p_full[HALF:N, :]
                    _gather(nc, g, src_ap, ixl_t, tg, T, 64)
                    gts.append((g, T))
                xr_g = gat.tile([128, T_all, 64], F32, tag="gxr")
                _gather(nc, xr_g, xr2p[:], ixr_t, tglob0, T_all, 64)
                u2 = sb.tile([128, T_all, OUT], F32, tag="u2")
                tloc = 0
                for g, T in gts:
                    nc.vector.tensor_tensor(out=u2[:, tloc:tloc + T, :], in0=g[:, :, 0:OUT],
                                            in1=xr_g[:, tloc:tloc + T, 0:OUT], op=ALU.add)
                    tloc += T
                v2 = sb.tile([128, T_all, OUT], F32, tag="v2")
                if PRELU_VIA_DVE:
                    nc.vector.tensor_scalar(out=v2[:], in0=u2[:], scalar1=NEG_SLOPE,
                                            scalar2=None, op0=ALU.mult)
                    nc.vector.tensor_tensor(out=v2[:], in0=v2[:], in1=u2[:], op=ALU.max)
                else:
                    nc.scalar.activation(v2[:], u2[:], AF.Prelu, alpha=NEG_SLOPE)
                p2_ = sb.tile([128, T_all, OUT], F32, tag="p2")
                nc.vector.tensor_tensor(out=p2_[:], in0=v2[:],
                                        in1=att2[:, None, :].to_broadcast([128, T_all, OUT]),
                                        op=ALU.mult)
                lg2 = sb.tile([128, T_all], F32, tag="lg2")
                nc.vector.tensor_reduce(out=lg2[:], in_=p2_[:], axis=mybir.AxisListType.X, op=ALU.add)
                ex2 = sb.tile([128, T_all], F32, tag="ex2")
                nc.scalar.activation(ex2[:], lg2[:], AF.Exp)
                acc2 = ps.tile([128, OUT + 1], F32, tag="acc", space="PSUM")
                tloc = 0
                for g, T in gts:
                    for t in range(T):
                        gt = tglob0 + tloc
                        oh = sb.tile([128, 128], F32, tag="oh")
                        nc.vector.tensor_scalar(out=oh[:], in0=cs("iota"),
                                                scalar1=dstl_t[:, gt:gt + 1], scalar2=None,
                                                op0=ALU.is_equal)
                        r2 = sb.tile([128, OUT + 1], F32, tag="r2")
                        nc.vector.tensor_tensor(
                            out=r2[:, 0:OUT], in0=g[:, t, 0:OUT],
                            in1=ex2[:, tloc:tloc + 1].to_broadcast([128, OUT]), op=ALU.mult)
                        nc.scalar.copy(r2[:, OUT:OUT + 1], ex2[:, tloc:tloc + 1])
                        nc.tensor.matmul(acc2[:], lhsT=oh[:], rhs=r2[:],
                                         start=(tloc == 0), stop=(tloc == T_all - 1))
                        tloc += 1
                rc2 = sb.tile([128, 1], F32, tag="rc2")
                nc.vector.reciprocal(rc2[:], acc2[:, OUT:OUT + 1])
                o2 = sb.tile([128, OUT], F32, tag="o2")
                nc.vector.tensor_scalar(out=o2[:], in0=acc2[:, 0:OUT], scalar1=rc2[:],
                                        scalar2=None, op0=ALU.mult)
                nc.vector.tensor_tensor(out=o_all[:, 2 * b:2 * b + 2], in0=o2[:],
                                        in1=cs("bias2B"), op=ALU.add)

            # ---------------- phase G: log_softmax ----------------
            for b in (range(NBLK) if MAX_PHASE >= 7 else []):
                st = b * BLOCK
                nreal = min(BLOCK, NPC - st)
                d = sb.tile([128, 1], F32, tag="d")
                nc.vector.tensor_tensor(out=d[:], in0=o_all[:, 2 * b + 1:2 * b + 2],
                                        in1=o_all[:, 2 * b:2 * b + 1], op=ALU.subtract)
                # lsm0 = -ln(1+e^d); lsm1 = d - ln(1+e^d)
                e = sb.tile([128, 1], F32, tag="e")
                nc.scalar.activation(e[:], d[:], AF.Exp)
                ep1 = sb.tile([128, 1], F32, tag="ep1")
                nc.vector.tensor_scalar(out=ep1[:], in0=e[:], scalar1=1.0, scalar2=None,
                                        op0=ALU.add)
                l = sb.tile([128, 1], F32, tag="l")
                nc.scalar.activation(l[:], ep1[:], AF.Ln)
                ls = sb.tile([128, 2], F32, tag="ls")
                nc.vector.tensor_scalar(out=ls[:, 0:1], in0=l[:], scalar1=-1.0, scalar2=None,
                                        op0=ALU.mult)
                nc.vector.tensor_tensor(out=ls[:, 1:2], in0=d[:], in1=l[:], op=ALU.subtract)
                nc.sync.dma_start(out[st:st + nreal, :], ls[:nreal, :])
            if MAX_PHASE < 7:
                dz = sb.tile([128, OUT], F32, tag="dz")
                nc.vector.memset(dz[:], 0.0)
                for st, sz in chunks:
                    nc.sync.dma_start(out[st:st + sz, :], dz[:sz, :])

    nc.compile()
    return nc


COLS = None  # set by kernel()


# ---------------------------------------------------------------- entry
_CACHE = {}
LAST_RESULTS = None


def kernel(**inputs):
    global COLS
    x = np.asarray(inputs["x"], np.float32)
    ei = np.asarray(inputs["edge_index"]).astype(np.int64)
    N = x.shape[0]
    NPC = N // NCORES

    idx_xl, idx_xr, dstl, schedule, NBLK = preprocess(ei, N)
    TT = sum(T for _, _, T in schedule)
    carr, COLS = pack_consts(inputs, N)
    CW = carr.shape[1]

    key = (N, TT, NBLK, tuple(schedule))
    if key not in _CACHE:
        _CACHE[key] = build(N, schedule, NBLK, TT, CW)
    nc = _CACHE[key]

    in_maps = []
    for c in range(NCORES):
        sl = slice(c * NPC, (c + 1) * NPC)
        in_maps.append(dict(
            xT=np.ascontiguousarray(x[sl].T),
            idx_xl=idx_xl[c], idx_xr=idx_xr[c], dstl=dstl[c], consts=carr,
        ))
    res = run_bass_kernel_spmd(nc, in_maps, list(range(NCORES)))
    global LAST_RESULTS
    LAST_RESULTS = res
    outs = [res.results[c]["out"] for c in range(NCORES)]
    return np.concatenate(outs, axis=0).astype(np.float32)



# revision 2
# speedup vs baseline: 1.3220x; 1.3220x over previous
"""Trainium2 Bass kernel for EnhancedGATModel (3-layer GATv2, N=50000, E=800000).

v2 design (8 cores, dst-partitioned graph):
- att magnitudes folded into Wl/Wr columns (host), columns permuted
  positive-att-first per head: per-edge logits become prefix-sum
  differences of leaky(xl''+xr'') -- one custom DVE scan op per 8-tile
  group, no per-edge multiply by att, no per-edge reduce.
- per-edge xr'' comes from a PE one-hot broadcast matmul out of an
  SBUF-resident per-block xr table (no xr gather).
- messages scattered TRANSPOSED ([channel, dst]) by PE with fp8 host
  one-hots; softmax denominator via a tiny per-tile matmul; per-dst
  reciprocal expanded back to [channel, dst] with a head-mask matmul.
- BN+relu fused into one ScalarE activation per 128-channel chunk
  (per-partition scale/bias absorb 1/|att| and the permutation).
- residual handled by a split matmul at the L2 node phase (h2+h1 both
  feed L2 tables) -- no elementwise residual add.
- bf16 tables/gathers, fp8 one-hots, 4 SWDGE queues for gather overlap.
"""
import sys
import numpy as np

sys.path.insert(0, "/opt/trn_rl_repo")

import ml_dtypes
import concourse.bass as bass
import concourse.mybir as mybir
import concourse.tile as tile
from concourse import bacc
from concourse import dve_ops as _dve_ops
from concourse.bass_utils import run_bass_kernel_spmd
from concourse.dve_spec import (
    Spec, Scan, Src0, Src1, C0, AluOp as SAlu, lower as dve_lower, maxx,
)
from concourse.dve_ops import DveOp
from concourse.dve_uop import DveOpSpec

F32 = mybir.dt.float32
BF16 = mybir.dt.bfloat16
F8 = mybir.dt.float8e4
I16 = mybir.dt.int16
AF = mybir.ActivationFunctionType
ALU = mybir.AluOpType
NPBF = ml_dtypes.bfloat16
NPF8 = ml_dtypes.float8_e4m3

N = 50000
NCORES = 8
NPC = N // NCORES          # 6250
BLOCK = 128
NBLK = (NPC + BLOCK - 1) // BLOCK   # 49
HALF = N // 2
D_IN, HID, HEADS, OUT = 128, 64, 4, 2
HC = HEADS * HID           # 256
NEG = 0.2
BN_EPS = 1e-5
G = 4                      # tiles per scan group (PSUM: [128, G, 256] f32 = 2 banks)
GMAX = 8                   # dma_gather <=1024 idxs per op


# ------------------------------------------------------------- custom DVE op
def _register_scan_op():
    """out = inclusive prefix sum (along free dims) of leaky_relu(in0+in1),
    slope s0."""
    name = "GAT_LEAKY_PREFIX_ANT"
    if name in _dve_ops._SUB_OPCODE_FOR_NAME:
        return next(o for o in _dve_ops.OPS if o.name == name)
    z = Src0 + Src1
    body = Scan(SAlu.ADD, maxx(z, z * C0))

    def _ref(in0, in1, s0, s1, imm2):
        zz = in0.astype(np.float32) + in1.astype(np.float32)
        lk = np.maximum(zz, zz * np.asarray(s0, np.float32).reshape(-1, 1)
                        if isinstance(s0, np.ndarray) else zz * s0)
        P = in0.shape[0]
        return np.add.accumulate(lk.reshape(P, -1), axis=1).reshape(in0.shape)

    spec = Spec(body=body, reference=_ref)
    shas = {}
    for ver in ("v3",):
        uops = dve_lower(spec, ver=ver)
        shas[ver] = DveOpSpec(name=name, opcode=0, uops=uops, rd1_en=True).sha(ver)
    op = DveOp(name, spec, subdim=False, uops_sha=shas)
    row = max(_dve_ops._SUB_OPCODE_FOR_NAME.values()) + 1
    assert row < 0x20
    _dve_ops.OPS.append(op)
    _dve_ops._SUB_OPCODE_FOR_NAME[name] = row
    _dve_ops.CUSTOM_DVE_SPECS[name] = spec
    return op


SCAN_OP = _register_scan_op()


# ---------------------------------------------------------------- host fold
def fold_att(att):
    att = np.asarray(att, np.float64)
    H, C = att.shape
    lam = np.abs(att)
    lam = np.maximum(lam, 1e-4 * max(lam.mean(), 1e-30))
    perm = np.zeros(H * C, np.int64)
    k = np.zeros(H, np.int64)
    for h in range(H):
        pos = np.where(att[h] >= 0)[0]
        neg = np.where(att[h] < 0)[0]
        k[h] = len(pos)
        perm[h * C:(h + 1) * C] = h * C + np.concatenate([pos, neg])
    return lam.reshape(-1), perm, k


def prep_weights(ip):
    f = {}
    lam0, p0, k0 = fold_att(np.asarray(ip["att0"], np.float64))
    lam1, p1, k1 = fold_att(np.asarray(ip["att1"], np.float64))
    lam2, p2, k2 = fold_att(np.asarray(ip["att2"], np.float64))
    W = lambda nm: np.asarray(ip[nm], np.float64)
    f["Wl0f"] = (W("Wl0") * lam0[None, :])[:, p0]
    f["Wr0f"] = (W("Wr0") * lam0[None, :])[:, p0]
    f["Wl1f"] = (W("Wl1") * lam1[None, :])[p0][:, p1]
    f["Wr1f"] = (W("Wr1") * lam1[None, :])[p0][:, p1]
    f["Wl2f_h2"] = (W("Wl2") * lam2[None, :])[p1][:, p2]
    f["Wl2f_h1"] = (W("Wl2") * lam2[None, :])[p0][:, p2]
    f["Wr2f_h2"] = (W("Wr2") * lam2[None, :])[p1][:, p2]
    f["Wr2f_h1"] = (W("Wr2") * lam2[None, :])[p0][:, p2]
    g = np.asarray(ip["bn_gamma"], np.float64); bt = np.asarray(ip["bn_beta"], np.float64)
    mu = np.asarray(ip["bn_mean"], np.float64); var = np.asarray(ip["bn_var"], np.float64)
    for l, (perm, lam) in enumerate(((p0, lam0), (p1, lam1))):
        a = g[l] / np.sqrt(var[l] + BN_EPS)
        b = bt[l] - mu[l] * a + a * np.asarray(ip[f"bias{l}"], np.float64)
        f[f"aT{l}"] = a[perm] / lam[perm]
        f[f"bT{l}"] = b[perm]
    f["rho2"] = np.argsort(p2)
    f["inv_lam2"] = 1.0 / lam2
    bias2 = np.asarray(ip["bias2"], np.float64)
    f["db2"] = float(bias2[1] - bias2[0])
    f["k0"], f["k1"], f["k2"] = k0, k1, int(k2[0])
    f["W_in"] = np.asarray(ip["W_in"], np.float64)
    f["b_in"] = np.asarray(ip["b_in"], np.float64)
    return f


# ---------------------------------------------------------------- host prep
def preprocess(edge_index):
    """Edge bucketing. Returns (idx, oh8, oh28, blk_runs, TT, Tmax).

    idx  [NCORES, 128, 8*TT] int16  gather indices (wrapped layout)
    oh8  [NCORES, 128, TT*128] f8   scatter one-hot, row=edge, col=dst
    oh28 [NCORES, 128, TT*128] f8   bcast one-hot, row=dst, col=edge
    blk_runs: per block list of (tg, half, T); uniform across cores.
    """
    src = np.concatenate([edge_index[0], np.arange(N)]).astype(np.int64)
    dst = np.concatenate([edge_index[1], np.arange(N)]).astype(np.int64)
    order = np.argsort(dst, kind="stable")
    src, dst = src[order], dst[order]
    core_of = dst // NPC
    groups = {}
    for c in range(NCORES):
        m = core_of == c
        sc, dc = src[m], dst[m] - c * NPC
        blk = dc // BLOCK
        for b in range(NBLK):
            mb = blk == b
            sb_, db_ = sc[mb], dc[mb] % BLOCK
            lo = sb_ < HALF
            groups[(c, b, 0)] = (sb_[lo], db_[lo])
            groups[(c, b, 1)] = (sb_[~lo] - HALF, db_[~lo])
    blk_runs = []
    TT = 0
    for b in range(NBLK):
        runs = []
        for h in (0, 1):
            mx = max(len(groups[(c, b, h)][0]) for c in range(NCORES))
            T = (mx + 127) // 128
            if T > 0:
                runs.append((TT, h, T))
                TT += T
        blk_runs.append(runs)
    Tmax = max(sum(T for _, _, T in runs) for runs in blk_runs)
    Trun = max(T for runs in blk_runs for _, _, T in runs)

    idx = np.zeros((NCORES, 128, 8 * TT), np.int16)
    dstl = np.full((NCORES, TT, 128), -1, np.int64)
    for b in range(NBLK):
        for (tg, h, T) in blk_runs[b]:
            for c in range(NCORES):
                s, dl = groups[(c, b, h)]
                ne = len(s)
                pad = T * 128 - ne
                sp = np.concatenate([s, np.zeros(pad, np.int64)])
                wrap = sp.reshape(8 * T, 16).T.astype(np.int16)
                idx[c, :, 8 * tg:8 * (tg + T)] = np.tile(wrap, (8, 1))
                dfull = np.concatenate([dl, np.full(pad, -1, np.int64)])
                dstl[c, tg:tg + T] = dfull.reshape(T, 128)
    # one-hots
    oh = np.zeros((NCORES, 128, TT, 128), np.float32)
    oh2 = np.zeros((NCORES, 128, TT, 128), np.float32)
    ci, ti, ei = np.where(dstl >= 0)
    dv = dstl[ci, ti, ei]
    oh[ci, ei, ti, dv] = 1.0
    oh2[ci, dv, ti, ei] = 1.0
    oh8 = oh.reshape(NCORES, 128, TT * 128).astype(NPF8)
    oh28 = oh2.reshape(NCORES, 128, TT * 128).astype(NPF8)
    return idx, oh8, oh28, blk_runs, TT, Tmax, Trun


def pack_consts(f):
    """Two packed const tensors: CB [128, *] bf16 and CF [128, *] f32."""
    bcols, bparts = {}, []
    fcols, fparts = {}, []

    def addb(name, arr):
        a = np.zeros((128, arr.shape[1]), np.float64)
        a[:arr.shape[0]] = arr
        bcols[name] = (arr.shape[0], sum(p.shape[1] for p in bparts), arr.shape[1])
        bparts.append(a)

    def addf(name, arr):
        a = np.zeros((128, arr.shape[1]), np.float64)
        a[:arr.shape[0]] = arr
        fcols[name] = (arr.shape[0], sum(p.shape[1] for p in fparts), arr.shape[1])
        fparts.append(a)

    addb("W_in", f["W_in"])                      # [128, 64]
    addb("Wl0f", f["Wl0f"]); addb("Wr0f", f["Wr0f"])          # [64, 256]
    addb("Wl1f0", f["Wl1f"][:128]); addb("Wl1f1", f["Wl1f"][128:])
    addb("Wr1f0", f["Wr1f"][:128]); addb("Wr1f1", f["Wr1f"][128:])
    for nm in ("Wl2f_h2", "Wl2f_h1", "Wr2f_h2", "Wr2f_h1"):
        addb(nm + "k0", f[nm][:128]); addb(nm + "k1", f[nm][128:])

    for l in (0, 1):
        addb(f"aT{l}", np.broadcast_to(f[f"aT{l}"][None, :], (128, HC)))
        addb(f"bT{l}", np.broadcast_to(f[f"bT{l}"][None, :], (128, HC)))
    addb("ident", np.eye(128))
    addf("b_in", f["b_in"].reshape(-1, 1))       # [64, 1]

    CB = np.concatenate(bparts, axis=1).astype(NPBF)
    CF = np.concatenate(fparts, axis=1).astype(np.float32)
    return CB, bcols, CF, fcols


# ---------------------------------------------------------------- device
def build(blk_runs, TT, Tmax, Trun, CBw, CFw, BCOLS, FCOLS, K0, K1, K2,
          RHO2, INV_LAM2, DB2):
    nc = bacc.Bacc("TRN2", target_bir_lowering=False, debug=False,
                   num_swdge_queues=4)

    xT = nc.dram_tensor("xT", [D_IN, NPC], BF16, kind="ExternalInput")
    idx = nc.dram_tensor("idx", [128, 8 * TT], I16, kind="ExternalInput")
    oh8 = nc.dram_tensor("oh8", [128, TT * 128], F8, kind="ExternalInput")
    oh28 = nc.dram_tensor("oh28", [128, TT * 128], F8, kind="ExternalInput")
    CB = nc.dram_tensor("CB", [128, CBw], BF16, kind="ExternalInput")
    CF = nc.dram_tensor("CF", [128, CFw], F32, kind="ExternalInput")
    out = nc.dram_tensor("out", [NPC, OUT], F32, kind="ExternalOutput")

    xl0_own = nc.dram_tensor("xl0_own", [NPC, HC], BF16)
    xl0_full = nc.dram_tensor("xl0_full", [N, HC], BF16, addr_space="Shared")
    h1T = nc.dram_tensor("h1T", [HC, NPC], BF16)
    xl1_own = nc.dram_tensor("xl1_own", [NPC, HC], BF16)
    xl1_full = nc.dram_tensor("xl1_full", [N, HC], BF16, addr_space="Shared")
    h2T = nc.dram_tensor("h2T", [HC, NPC], BF16)
    xl2_own = nc.dram_tensor("xl2_own", [NPC, 128], BF16)
    xl2_full = nc.dram_tensor("xl2_full", [N, 128], BF16, addr_space="Shared")

    rg = [list(range(NCORES))]
    qn = [0]

    def next_q():
        qn[0] = (qn[0] + 1) % 4
        return qn[0]

    with tile.TileContext(nc) as tc:
        import contextlib
        with contextlib.ExitStack() as ctx:
            cst = ctx.enter_context(tc.tile_pool(name="cst", bufs=1))
            sb = ctx.enter_context(tc.tile_pool(name="sb", bufs=3))
            gat = ctx.enter_context(tc.tile_pool(name="gat", bufs=6))
            scr = ctx.enter_context(tc.tile_pool(name="scr", bufs=2))
            ps_pxr = ctx.enter_context(tc.tile_pool(name="pspxr", bufs=2, space="PSUM"))
            ps_acc = ctx.enter_context(tc.tile_pool(name="psacc", bufs=2, space="PSUM"))
            ps_misc = ctx.enter_context(tc.tile_pool(name="psmisc", bufs=2, space="PSUM"))

            CBt = cst.tile([128, CBw], BF16)
            nc.sync.dma_start(CBt[:], CB[:])
            CFt = cst.tile([128, CFw], F32)
            nc.sync.dma_start(CFt[:], CF[:])
            idx_t = cst.tile([128, 8 * TT], I16)
            nc.sync.dma_start(idx_t[:], idx[:])
            xrA = cst.tile([128, NBLK * HC], BF16)    # L0 xr'' table
            xrB = cst.tile([128, NBLK * HC], BF16)    # L1 xr'' table
            xr2 = cst.tile([128, NBLK * OUT], BF16)   # L2 xr'' table

            def cb(name):
                r, c0, w = BCOLS[name]
                return CBt[0:r, c0:c0 + w]

            def cf(name):
                r, c0, w = FCOLS[name]
                return CFt[0:r, c0:c0 + w]

            chunks = [(i * 128, min(128, NPC - i * 128)) for i in range(NBLK)]

            # ================= phase A: L0 node tables =================
            for ci, (st, sz) in enumerate(chunks):
                xTc = sb.tile([D_IN, 128], BF16, tag="xTc")
                nc.sync.dma_start(xTc[:, :sz], xT[:, st:st + sz])
                p0 = ps_misc.tile([64, 128], F32, tag="misc")
                nc.tensor.matmul(p0[:, :sz], lhsT=cb("W_in"), rhs=xTc[:, :sz],
                                 start=True, stop=True)
                h0T = sb.tile([64, 128], BF16, tag="h0T")
                nc.scalar.activation(h0T[:, :sz], p0[:, :sz], AF.Relu,
                                     bias=cf("b_in"))
                for Wn, isl in (("Wl0f", False), ("Wr0f", True)):
                    p1 = ps_misc.tile([128, HC], F32, tag="misc")
                    nc.tensor.matmul(p1[:sz, :], lhsT=h0T[:, :sz], rhs=cb(Wn),
                                     start=True, stop=True)
                    if isl:
                        dstap = xrA[:, ci * HC:(ci + 1) * HC]
                        if sz < 128:
                            nc.vector.memset(dstap, 0.0)
                        nc.vector.tensor_copy(dstap[:sz, :], p1[:sz, :])
                    else:
                        cp = sb.tile([128, HC], BF16, tag="cp")
                        nc.vector.tensor_copy(cp[:sz, :], p1[:sz, :])
                        nc.sync.dma_start(xl0_own[st:st + sz, :], cp[:sz, :])

            nc.gpsimd.collective_compute(
                "AllGather", ALU.bypass, ins=[xl0_own[:]], outs=[xl0_full[:]],
                replica_groups=rg)

            # ================= shared edge pass (layers 0/1) =================
            def edge_pass(xl_full, xr_res, Ks, aTn, bTn, hT_dst):
                for b in range(NBLK):
                    runs = blk_runs[b]
                    T_all = sum(T for _, _, T in runs)
                    tg0 = runs[0][0]
                    st = b * BLOCK
                    nreal = min(BLOCK, NPC - st)
                    xr_ap = xr_res[:, b * HC:(b + 1) * HC]

                    ohb = sb.tile([128, Tmax * 128], F8, tag="ohb")
                    nc.sync.dma_start(ohb[:, :T_all * 128],
                                      oh8[:, tg0 * 128:(tg0 + T_all) * 128])
                    oh2b = sb.tile([128, Tmax * 128], F8, tag="oh2b")
                    nc.sync.dma_start(oh2b[:, :T_all * 128],
                                      oh28[:, tg0 * 128:(tg0 + T_all) * 128])

                    Pscr = scr.tile([128, 257 + Tmax * HC], F32, tag="scr")
                    nc.vector.memset(Pscr[:, 0:1], 0.0)

                    gts = []
                    for (tg, hlf, T) in runs:
                        g = gat.tile([128, Trun, HC], BF16, tag="g")
                        src_ap = xl_full[0:HALF, :] if hlf == 0 else xl_full[HALF:N, :]
                        k = 0
                        while k < T:
                            Tc = min(GMAX, T - k)
                            nc.gpsimd.dma_gather(
                                out_ap=g[:, k:k + Tc, :], in_ap=src_ap,
                                idxs_ap=idx_t[:, 8 * (tg + k):8 * (tg + k + Tc)],
                                num_idxs=128 * Tc, num_idxs_reg=128 * Tc,
                                elem_size=HC, queue_num=next_q())
                            k += Tc
                        gts.append((g, T))

                    # scans (leaky prefix) per group
                    tb = 0
                    for (g, T) in gts:
                        for gk in range(0, T, G):
                            Gc = min(G, T - gk)
                            pxr = ps_pxr.tile([128, G, HC], F32, tag="pxr")
                            for t in range(Gc):
                                tbt = tb + gk + t
                                nc.tensor.matmul(
                                    pxr[:, t, :],
                                    lhsT=oh2b[:, tbt * 128:(tbt + 1) * 128],
                                    rhs=xr_ap, start=True, stop=True)
                            o_ap = Pscr[:, 1 + (tb + gk) * HC:1 + (tb + gk + Gc) * HC]
                            nc.vector._custom_dve(
                                SCAN_OP,
                                out=o_ap.rearrange("p (t c) -> p t c", c=HC),
                                in0=g[:, gk:gk + Gc, :],
                                in1=pxr[:, 0:Gc, :], s0=NEG)
                        tb += T

                    # logits via boundary extraction
                    lgf = sb.tile([128, Tmax, HEADS], F32, tag="lgf")
                    for h in range(HEADS):
                        c1 = 64 * h
                        c2 = 64 * h + int(Ks[h])
                        ap2 = Pscr[:, c2:c2 + T_all * HC].rearrange(
                            "p (t o) -> p t o", o=HC)[:, :, 0]
                        ap1 = Pscr[:, c1:c1 + T_all * HC].rearrange(
                            "p (t o) -> p t o", o=HC)[:, :, 0]
                        nc.vector.scalar_tensor_tensor(
                            out=lgf[:, 0:T_all, h], in0=ap2, scalar=2.0,
                            in1=ap1, op0=ALU.mult, op1=ALU.subtract)
                    ap3 = Pscr[:, 64:64 + T_all * HC].rearrange(
                        "p (t h c) -> p t h c", h=HEADS, c=64)[:, :, :, 0]
                    nc.vector.tensor_tensor(out=lgf[:, 0:T_all, :],
                                            in0=lgf[:, 0:T_all, :], in1=ap3,
                                            op=ALU.subtract)

                    # weighted rhs (cols 0:256 = g*ex, 256:260 = ex) + scatter
                    acc = ps_acc.tile([128, 260], F32, tag="acc")
                    tb = 0
                    for (g, T) in gts:
                        for gk in range(0, T, G):
                            Gc = min(G, T - gk)
                            rhs = sb.tile([128, G, 260], BF16, tag="rhs")
                            nc.scalar.activation(rhs[:, 0:Gc, HC:HC + HEADS],
                                                 lgf[:, tb + gk:tb + gk + Gc, :],
                                                 AF.Exp)
                            nc.vector.tensor_tensor(
                                out=rhs[:, 0:Gc, 0:HC].rearrange(
                                    "p t (h c) -> p t h c", h=HEADS),
                                in0=g[:, gk:gk + Gc, :].rearrange(
                                    "p t (h c) -> p t h c", h=HEADS),
                                in1=rhs[:, 0:Gc, HC:HC + HEADS][:, :, :, None]
                                .to_broadcast([128, Gc, HEADS, 64]),
                                op=ALU.mult)
                            for t in range(Gc):
                                tbt = tb + gk + t
                                nc.tensor.matmul(
                                    acc[:], lhsT=ohb[:, tbt * 128:(tbt + 1) * 128],
                                    rhs=rhs[:, t, :],
                                    start=(tbt == 0), stop=(tbt == T_all - 1))
                        tb += T

                    # block post: normalize + BN + relu + transpose out
                    rc4 = sb.tile([128, HEADS], F32, tag="rc4")
                    nc.vector.reciprocal_approx_fast(rc4[:], acc[:, HC:HC + HEADS])
                    m1 = sb.tile([128, HC], BF16, tag="m1")
                    nc.vector.tensor_tensor(
                        out=m1[:].rearrange("p (h c) -> p h c", h=HEADS),
                        in0=acc[:, 0:HC].rearrange("p (h c) -> p h c", h=HEADS),
                        in1=rc4[:, :, None].to_broadcast([128, HEADS, 64]),
                        op=ALU.mult)
                    m2 = sb.tile([128, HC], BF16, tag="m2")
                    nc.vector.tensor_tensor(out=m2[:], in0=m1[:], in1=cb(aTn),
                                            op=ALU.mult)
                    nc.vector.tensor_tensor(out=m2[:], in0=m2[:], in1=cb(bTn),
                                            op=ALU.add)
                    h_ = sb.tile([128, HC], BF16, tag="h_")
                    nc.vector.tensor_scalar(out=h_[:], in0=m2[:], scalar1=0.0,
                                            scalar2=None, op0=ALU.max)
                    for kk in range(2):
                        tp = ps_misc.tile([128, 128], BF16, tag="misc")
                        nc.tensor.transpose(tp[:], h_[:, kk * 128:(kk + 1) * 128],
                                            cb("ident"))
                        hk = sb.tile([128, 128], BF16, tag="hk")
                        nc.scalar.copy(hk[:], tp[:])
                        nc.sync.dma_start(
                            hT_dst[kk * 128:(kk + 1) * 128, st:st + nreal],
                            hk[:, :nreal])

            edge_pass(xl0_full, xrA, K0, "aT0", "bT0", h1T)

            # ================= phase C: L1 node tables =================
            for ci, (st, sz) in enumerate(chunks):
                ht0 = sb.tile([128, 128], BF16, tag="ht0")
                nc.sync.dma_start(ht0[:, :sz], h1T[0:128, st:st + sz])
                ht1 = sb.tile([128, 128], BF16, tag="ht1")
                nc.sync.dma_start(ht1[:, :sz], h1T[128:256, st:st + sz])
                for (W0n, W1n, isl) in (("Wl1f0", "Wl1f1", False),
                                        ("Wr1f0", "Wr1f1", True)):
                    p1 = ps_misc.tile([128, HC], F32, tag="misc")
                    nc.tensor.matmul(p1[:sz, :], lhsT=ht0[:, :sz], rhs=cb(W0n),
                                     start=True, stop=False)
                    nc.tensor.matmul(p1[:sz, :], lhsT=ht1[:, :sz], rhs=cb(W1n),
                                     start=False, stop=True)
                    if isl:
                        dstap = xrB[:, ci * HC:(ci + 1) * HC]
                        if sz < 128:
                            nc.vector.memset(dstap, 0.0)
                        nc.vector.tensor_copy(dstap[:sz, :], p1[:sz, :])
                    else:
                        cp = sb.tile([128, HC], BF16, tag="cp")
                        nc.vector.tensor_copy(cp[:sz, :], p1[:sz, :])
                        nc.sync.dma_start(xl1_own[st:st + sz, :], cp[:sz, :])

            nc.gpsimd.collective_compute(
                "AllGather", ALU.bypass, ins=[xl1_own[:]], outs=[xl1_full[:]],
                replica_groups=rg)

            edge_pass(xl1_full, xrB, K1, "aT1", "bT1", h2T)

            # ================= phase E: L2 node tables =================
            for ci, (st, sz) in enumerate(chunks):
                h2t0 = sb.tile([128, 128], BF16, tag="ht0")
                nc.sync.dma_start(h2t0[:, :sz], h2T[0:128, st:st + sz])
                h2t1 = sb.tile([128, 128], BF16, tag="ht1")
                nc.sync.dma_start(h2t1[:, :sz], h2T[128:256, st:st + sz])
                h1t0 = sb.tile([128, 128], BF16, tag="h1t0")
                nc.sync.dma_start(h1t0[:, :sz], h1T[0:128, st:st + sz])
                h1t1 = sb.tile([128, 128], BF16, tag="h1t1")
                nc.sync.dma_start(h1t1[:, :sz], h1T[128:256, st:st + sz])
                for base, isl in (("Wl2f", False), ("Wr2f", True)):
                    p1 = ps_misc.tile([128, OUT], F32, tag="misc")
                    nc.tensor.matmul(p1[:sz, :], lhsT=h2t0[:, :sz],
                                     rhs=cb(base + "_h2k0"), start=True, stop=False)
                    nc.tensor.matmul(p1[:sz, :], lhsT=h2t1[:, :sz],
                                     rhs=cb(base + "_h2k1"), start=False, stop=False)
                    nc.tensor.matmul(p1[:sz, :], lhsT=h1t0[:, :sz],
                                     rhs=cb(base + "_h1k0"), start=False, stop=False)
                    nc.tensor.matmul(p1[:sz, :], lhsT=h1t1[:, :sz],
                                     rhs=cb(base + "_h1k1"), start=False, stop=True)
                    if isl:
                        dstap = xr2[:, ci * OUT:(ci + 1) * OUT]
                        if sz < 128:
                            nc.vector.memset(dstap, 0.0)
                        nc.vector.tensor_copy(dstap[:sz, :], p1[:sz, :])
                    else:
                        cp2 = sb.tile([128, 128], BF16, tag="cp2")
                        nc.vector.memset(cp2[:], 0.0)
                        nc.vector.tensor_copy(cp2[:sz, 0:OUT], p1[:sz, :])
                        nc.sync.dma_start(xl2_own[st:st + sz, :], cp2[:sz, :])

            nc.gpsimd.collective_compute(
                "AllGather", ALU.bypass, ins=[xl2_own[:]], outs=[xl2_full[:]],
                replica_groups=rg)

            # ================= phase F: L2 edge + log_softmax =================
            dlt_all = cst.tile([128, NBLK], F32)
            for b in range(NBLK):
                runs = blk_runs[b]
                T_all = sum(T for _, _, T in runs)
                tg0 = runs[0][0]
                st = b * BLOCK
                nreal = min(BLOCK, NPC - st)
                xr_ap = xr2[:, b * OUT:(b + 1) * OUT]

                ohb = sb.tile([128, Tmax * 128], F8, tag="ohb")
                nc.sync.dma_start(ohb[:, :T_all * 128],
                                  oh8[:, tg0 * 128:(tg0 + T_all) * 128])
                oh2b = sb.tile([128, Tmax * 128], F8, tag="oh2b")
                nc.sync.dma_start(oh2b[:, :T_all * 128],
                                  oh28[:, tg0 * 128:(tg0 + T_all) * 128])

                Pscr = scr.tile([128, 4 + Tmax * OUT], F32, tag="scr2")
                nc.vector.memset(Pscr[:, 0:1], 0.0)

                gts = []
                for (tg, hlf, T) in runs:
                    g = gat.tile([128, Trun, 128], BF16, tag="g2")
                    src_ap = xl2_full[0:HALF, :] if hlf == 0 else xl2_full[HALF:N, :]
                    k = 0
                    while k < T:
                        Tc = min(GMAX, T - k)
                        nc.gpsimd.dma_gather(
                            out_ap=g[:, k:k + Tc, :], in_ap=src_ap,
                            idxs_ap=idx_t[:, 8 * (tg + k):8 * (tg + k + Tc)],
                            num_idxs=128 * Tc, num_idxs_reg=128 * Tc,
                            elem_size=128, queue_num=next_q())
                        k += Tc
                    gts.append((g, T))

                tb = 0
                for (g, T) in gts:
                    for gk in range(0, T, G):
                        Gc = min(G, T - gk)
                        pxr = ps_pxr.tile([128, G, OUT], F32, tag="pxr")
                        for t in range(Gc):
                            tbt = tb + gk + t
                            nc.tensor.matmul(
                                pxr[:, t, :],
                                lhsT=oh2b[:, tbt * 128:(tbt + 1) * 128],
                                rhs=xr_ap, start=True, stop=True)
                        o_ap = Pscr[:, 1 + (tb + gk) * OUT:1 + (tb + gk + Gc) * OUT]
                        nc.vector._custom_dve(
                            SCAN_OP,
                            out=o_ap.rearrange("p (t c) -> p t c", c=OUT),
                            in0=g[:, gk:gk + Gc, 0:OUT],
                            in1=pxr[:, 0:Gc, :], s0=NEG)
                    tb += T

                lg2 = sb.tile([128, Tmax], F32, tag="lg2")
                ap2 = Pscr[:, K2:K2 + T_all * OUT].rearrange(
                    "p (t o) -> p t o", o=OUT)[:, :, 0]
                ap1 = Pscr[:, 0:T_all * OUT].rearrange(
                    "p (t o) -> p t o", o=OUT)[:, :, 0]
                nc.vector.scalar_tensor_tensor(
                    out=lg2[:, 0:T_all], in0=ap2, scalar=2.0, in1=ap1,
                    op0=ALU.mult, op1=ALU.subtract)
                ap3 = Pscr[:, OUT:OUT + T_all * OUT].rearrange(
                    "p (t o) -> p t o", o=OUT)[:, :, 0]
                nc.vector.tensor_tensor(out=lg2[:, 0:T_all], in0=lg2[:, 0:T_all],
                                        in1=ap3, op=ALU.subtract)
                rhs2 = sb.tile([128, Tmax, 3], BF16, tag="rhs2")
                nc.scalar.activation(rhs2[:, 0:T_all, 2], lg2[:, 0:T_all], AF.Exp)
                tb = 0
                for (g, T) in gts:
                    nc.vector.tensor_tensor(
                        out=rhs2[:, tb:tb + T, 0:OUT], in0=g[:, 0:T, 0:OUT],
                        in1=rhs2[:, tb:tb + T, 2:3].to_broadcast([128, T, OUT]),
                        op=ALU.mult)
                    tb += T
                acc2 = ps_acc.tile([128, 3], F32, tag="acc")
                for tbt in range(T_all):
                    nc.tensor.matmul(acc2[:],
                                     lhsT=ohb[:, tbt * 128:(tbt + 1) * 128],
                                     rhs=rhs2[:, tbt, :],
                                     start=(tbt == 0), stop=(tbt == T_all - 1))
                rc2 = sb.tile([128, 1], F32, tag="rc2")
                nc.vector.reciprocal_approx_fast(rc2[:], acc2[:, 2:3])
                v0 = sb.tile([128, 1], F32, tag="v0")
                nc.vector.tensor_scalar(out=v0[:], in0=acc2[:, RHO2[0]:RHO2[0] + 1],
                                        scalar1=float(INV_LAM2[0]), scalar2=None,
                                        op0=ALU.mult)
                d0 = sb.tile([128, 1], F32, tag="d0")
                nc.vector.scalar_tensor_tensor(
                    out=d0[:], in0=acc2[:, RHO2[1]:RHO2[1] + 1],
                    scalar=float(INV_LAM2[1]), in1=v0[:],
                    op0=ALU.mult, op1=ALU.subtract)
                nc.vector.tensor_scalar(out=dlt_all[:, b:b + 1], in0=d0[:],
                                        scalar1=rc2[:], scalar2=DB2,
                                        op0=ALU.mult, op1=ALU.add)

            # batched softplus tail: one exp + one ln for all blocks
            eall = cst.tile([128, NBLK], F32)
            nc.scalar.activation(eall[:], dlt_all[:], AF.Exp)
            nc.vector.tensor_scalar(out=eall[:], in0=eall[:], scalar1=1.0,
                                    scalar2=None, op0=ALU.add)
            lall = cst.tile([128, NBLK], F32)
            nc.scalar.activation(lall[:], eall[:], AF.Ln)
            for b in range(NBLK):
                st = b * BLOCK
                nreal = min(BLOCK, NPC - st)
                ls = sb.tile([128, 2], F32, tag="ls")
                nc.vector.tensor_scalar(out=ls[:, 0:1], in0=lall[:, b:b + 1],
                                        scalar1=-1.0, scalar2=None, op0=ALU.mult)
                nc.vector.tensor_tensor(out=ls[:, 1:2], in0=dlt_all[:, b:b + 1],
                                        in1=lall[:, b:b + 1], op=ALU.subtract)
                nc.sync.dma_start(out[st:st + nreal, :], ls[:nreal, :])

    nc.compile()
    return nc


# ---------------------------------------------------------------- entry
_CACHE = {}
_PREP_CACHE = {}
LAST_RESULTS = None


def kernel(**inputs):
    global LAST_RESULTS
    import hashlib
    x = np.asarray(inputs["x"], np.float32)
    ei = np.asarray(inputs["edge_index"]).astype(np.int64)

    f = prep_weights(inputs)
    dig = hashlib.blake2b(ei.tobytes(), digest_size=16).hexdigest()
    if dig not in _PREP_CACHE:
        _PREP_CACHE[dig] = preprocess(ei)
    idx, oh8, oh28, blk_runs, TT, Tmax, Trun = _PREP_CACHE[dig]
    CBa, BCOLS, CFa, FCOLS = pack_consts(f)

    key = (TT, Tmax, tuple(tuple(r) for rs in blk_runs for r in rs),
           tuple(f["k0"]), tuple(f["k1"]), f["k2"], CBa.shape[1], CFa.shape[1],
           tuple(f["rho2"]))
    if key not in _CACHE:
        _CACHE[key] = build(blk_runs, TT, Tmax, Trun, CBa.shape[1], CFa.shape[1],
                            BCOLS, FCOLS, f["k0"], f["k1"], f["k2"],
                            f["rho2"], f["inv_lam2"], f["db2"])
    nc = _CACHE[key]

    in_maps = []
    for c in range(NCORES):
        sl = slice(c * NPC, (c + 1) * NPC)
        in_maps.append(dict(
            xT=np.ascontiguousarray(x[sl].T).astype(NPBF),
            idx=idx[c], oh8=oh8[c], oh28=oh28[c],
            CB=CBa, CF=CFa,
        ))
    res = run_bass_kernel_spmd(nc, in_maps, list(range(NCORES)))
    LAST_RESULTS = res
    outs = [res.results[c]["out"] for c in range(NCORES)]
    return np.concatenate(outs, axis=0).astype(np.float32)


# revision 5
# speedup vs baseline: 1.3422x; 1.0152x over previous
"""Trainium2 Bass kernel for EnhancedGATModel (3-layer GATv2, N=50000, E=800000).

v2 design (8 cores, dst-partitioned graph):
- att magnitudes folded into Wl/Wr columns (host), columns permuted
  positive-att-first per head: per-edge logits become prefix-sum
  differences of leaky(xl''+xr'') -- one custom DVE scan op per 8-tile
  group, no per-edge multiply by att, no per-edge reduce.
- per-edge xr'' comes from a PE one-hot broadcast matmul out of an
  SBUF-resident per-block xr table (no xr gather).
- messages scattered TRANSPOSED ([channel, dst]) by PE with fp8 host
  one-hots; softmax denominator via a tiny per-tile matmul; per-dst
  reciprocal expanded back to [channel, dst] with a head-mask matmul.
- BN+relu fused into one ScalarE activation per 128-channel chunk
  (per-partition scale/bias absorb 1/|att| and the permutation).
- residual handled by a split matmul at the L2 node phase (h2+h1 both
  feed L2 tables) -- no elementwise residual add.
- bf16 tables/gathers, fp8 one-hots, 4 SWDGE queues for gather overlap.
"""
import sys
import numpy as np

sys.path.insert(0, "/opt/trn_rl_repo")

import ml_dtypes
import concourse.bass as bass
import concourse.mybir as mybir
import concourse.tile as tile
from concourse import bacc
from concourse import dve_ops as _dve_ops
from concourse.bass_utils import run_bass_kernel_spmd
from concourse.dve_spec import (
    Spec, Scan, Src0, Src1, C0, AluOp as SAlu, lower as dve_lower, maxx,
)
from concourse.dve_ops import DveOp
from concourse.dve_uop import DveOpSpec

F32 = mybir.dt.float32
BF16 = mybir.dt.bfloat16
F8 = mybir.dt.float8e4
I16 = mybir.dt.int16
AF = mybir.ActivationFunctionType
ALU = mybir.AluOpType
NPBF = ml_dtypes.bfloat16
NPF8 = ml_dtypes.float8_e4m3

N = 50000
NCORES = 8
NPC = N // NCORES          # 6250
BLOCK = 128
NBLK = (NPC + BLOCK - 1) // BLOCK   # 49
HALF = N // 2
D_IN, HID, HEADS, OUT = 128, 64, 4, 2
HC = HEADS * HID           # 256
NEG = 0.2
BN_EPS = 1e-5
G = 4                      # tiles per scan group (PSUM: [128, G, 256] f32 = 2 banks)
GMAX = 8                   # dma_gather <=1024 idxs per op


# ------------------------------------------------------------- custom DVE op
def _register_scan_op():
    """out = inclusive prefix sum (along free dims) of leaky_relu(in0+in1),
    slope s0."""
    name = "GAT_LEAKY_PREFIX_ANT"
    if name in _dve_ops._SUB_OPCODE_FOR_NAME:
        return next(o for o in _dve_ops.OPS if o.name == name)
    z = Src0 + Src1
    body = Scan(SAlu.ADD, maxx(z, z * C0))

    def _ref(in0, in1, s0, s1, imm2):
        zz = in0.astype(np.float32) + in1.astype(np.float32)
        lk = np.maximum(zz, zz * np.asarray(s0, np.float32).reshape(-1, 1)
                        if isinstance(s0, np.ndarray) else zz * s0)
        P = in0.shape[0]
        return np.add.accumulate(lk.reshape(P, -1), axis=1).reshape(in0.shape)

    spec = Spec(body=body, reference=_ref)
    shas = {}
    for ver in ("v3",):
        uops = dve_lower(spec, ver=ver)
        shas[ver] = DveOpSpec(name=name, opcode=0, uops=uops, rd1_en=True).sha(ver)
    op = DveOp(name, spec, subdim=False, uops_sha=shas)
    row = max(_dve_ops._SUB_OPCODE_FOR_NAME.values()) + 1
    assert row < 0x20
    _dve_ops.OPS.append(op)
    _dve_ops._SUB_OPCODE_FOR_NAME[name] = row
    _dve_ops.CUSTOM_DVE_SPECS[name] = spec
    return op


SCAN_OP = _register_scan_op()


# ---------------------------------------------------------------- host fold
def fold_att(att):
    att = np.asarray(att, np.float64)
    H, C = att.shape
    lam = np.abs(att)
    lam = np.maximum(lam, 1e-4 * max(lam.mean(), 1e-30))
    perm = np.zeros(H * C, np.int64)
    k = np.zeros(H, np.int64)
    for h in range(H):
        pos = np.where(att[h] >= 0)[0]
        neg = np.where(att[h] < 0)[0]
        k[h] = len(pos)
        perm[h * C:(h + 1) * C] = h * C + np.concatenate([pos, neg])
    return lam.reshape(-1), perm, k


def prep_weights(ip):
    f = {}
    lam0, p0, k0 = fold_att(np.asarray(ip["att0"], np.float64))
    lam1, p1, k1 = fold_att(np.asarray(ip["att1"], np.float64))
    lam2, p2, k2 = fold_att(np.asarray(ip["att2"], np.float64))
    W = lambda nm: np.asarray(ip[nm], np.float64)
    f["Wl0f"] = (W("Wl0") * lam0[None, :])[:, p0]
    f["Wr0f"] = (W("Wr0") * lam0[None, :])[:, p0]
    f["Wl1f"] = (W("Wl1") * lam1[None, :])[p0][:, p1]
    f["Wr1f"] = (W("Wr1") * lam1[None, :])[p0][:, p1]
    f["Wl2f_h2"] = (W("Wl2") * lam2[None, :])[p1][:, p2]
    f["Wl2f_h1"] = (W("Wl2") * lam2[None, :])[p0][:, p2]
    f["Wr2f_h2"] = (W("Wr2") * lam2[None, :])[p1][:, p2]
    f["Wr2f_h1"] = (W("Wr2") * lam2[None, :])[p0][:, p2]
    g = np.asarray(ip["bn_gamma"], np.float64); bt = np.asarray(ip["bn_beta"], np.float64)
    mu = np.asarray(ip["bn_mean"], np.float64); var = np.asarray(ip["bn_var"], np.float64)
    for l, (perm, lam) in enumerate(((p0, lam0), (p1, lam1))):
        a = g[l] / np.sqrt(var[l] + BN_EPS)
        b = bt[l] - mu[l] * a + a * np.asarray(ip[f"bias{l}"], np.float64)
        f[f"aT{l}"] = a[perm] / lam[perm]
        f[f"bT{l}"] = b[perm]
    f["rho2"] = np.argsort(p2)
    f["inv_lam2"] = 1.0 / lam2
    bias2 = np.asarray(ip["bias2"], np.float64)
    f["db2"] = float(bias2[1] - bias2[0])
    f["k0"], f["k1"], f["k2"] = k0, k1, int(k2[0])
    f["W_in"] = np.asarray(ip["W_in"], np.float64)
    f["b_in"] = np.asarray(ip["b_in"], np.float64)
    return f


# ---------------------------------------------------------------- host prep
def _balance_nodes(src_all, dst_all):
    """Assign nodes to (core, block) bins so that every bin's lo-half and
    hi-half edge loads are both balanced (the schedule pads each (block,
    half) to the max over cores).  Phase 1 splits nodes into the two table
    halves (cores 0-3 vs 4-7) by degree; phase 2 deals nodes into bins of
    their half, balancing the then-known per-half in-edge loads."""
    import heapq
    deg = np.bincount(dst_all, minlength=N)
    order = np.argsort(-deg, kind="stable")
    # phase 1: alternate nodes (by degree) between the two halves
    in_lo = np.zeros(N, bool)
    in_lo[order[0::2]] = True
    if in_lo.sum() != HALF // NPC * NPC // 1 and in_lo.sum() != N // 2:
        pass
    # force exact N/2 membership
    lo_ids = order[0::2][:N // 2]
    in_lo[:] = False
    in_lo[lo_ids] = True
    if in_lo.sum() < N // 2:
        extra = np.where(~in_lo)[0][:N // 2 - in_lo.sum()]
        in_lo[extra] = True
    # per-node per-half in-degree (src half known now)
    lo_d = np.bincount(dst_all[in_lo[src_all]], minlength=N)
    hi_d = deg - lo_d
    # phase 2: deal within each half: 4 cores x NBLK bins
    node_list = [None] * NCORES
    for half, cores in ((True, (0, 1, 2, 3)), (False, (4, 5, 6, 7))):
        ids = np.where(in_lo == half)[0]
        ids = ids[np.argsort(-(deg[ids]), kind="stable")]
        cap = {(c, b): (BLOCK if b < NBLK - 1 else NPC - (NBLK - 1) * BLOCK)
               for c in cores for b in range(NBLK)}
        fill = {k: 0 for k in cap}
        nodes = {k: [] for k in cap}
        heap = [(0, 0, c, b) for c in cores for b in range(NBLK)]
        heapq.heapify(heap)
        for nd in ids:
            while True:
                mx, mn, c, b = heapq.heappop(heap)
                if fill[(c, b)] < cap[(c, b)]:
                    break
            nodes[(c, b)].append(nd)
            fill[(c, b)] += 1
            lo2 = -mn if False else 0
            # track (lo_load, hi_load) via encoded tuple
            # recompute loads stored alongside:
            nodes.setdefault((c, b, "lo"), 0)
            nodes.setdefault((c, b, "hi"), 0)
            nodes[(c, b, "lo")] += int(lo_d[nd])
            nodes[(c, b, "hi")] += int(hi_d[nd])
            if fill[(c, b)] < cap[(c, b)]:
                l_, h_ = nodes[(c, b, "lo")], nodes[(c, b, "hi")]
                heapq.heappush(heap, (max(l_, h_), min(l_, h_), c, b))
        for c in cores:
            # order bins so heavy bins share a block index across cores
            # (schedule pads each index to the max over cores); the partial
            # bin must stay at index NBLK-1 (device uses nreal there).
            full = sorted(range(NBLK - 1),
                          key=lambda b: max(nodes[(c, b, "lo")],
                                            nodes[(c, b, "hi")]))
            border = full + [NBLK - 1]
            node_list[c] = np.array(
                sum((nodes[(c, b)] for b in border), []), np.int64)
    pos = np.zeros(N, np.int64)
    for c in range(NCORES):
        pos[node_list[c]] = c * NPC + np.arange(NPC)
    return node_list, pos


def preprocess(edge_index):
    """Edge bucketing with degree-balanced node placement.

    idx  [NCORES, 128, 8*TT] int16  gather indices (wrapped layout)
    oh8  [NCORES, 128, TT*128] f8   scatter one-hot, row=edge, col=dst
    oh28 [NCORES, 128, TT*128] f8   bcast one-hot, row=dst, col=edge
    blk_runs: per block list of (tg, half, T); uniform across cores.
    """
    src0 = np.concatenate([edge_index[0], np.arange(N)]).astype(np.int64)
    dst0 = np.concatenate([edge_index[1], np.arange(N)]).astype(np.int64)
    node_list, pos = _balance_nodes(src0, dst0)
    src = pos[src0]
    dst = pos[dst0]
    order = np.argsort(dst, kind="stable")
    src, dst = src[order], dst[order]
    core_of = dst // NPC
    groups = {}
    for c in range(NCORES):
        m = core_of == c
        sc, dc = src[m], dst[m] - c * NPC
        blk = dc // BLOCK
        for b in range(NBLK):
            mb = blk == b
            sb_, db_ = sc[mb], dc[mb] % BLOCK
            lo = sb_ < HALF
            groups[(c, b, 0)] = (sb_[lo], db_[lo])
            groups[(c, b, 1)] = (sb_[~lo] - HALF, db_[~lo])
    blk_runs = []
    TT = 0
    for b in range(NBLK):
        runs = []
        for h in (0, 1):
            mx = max(len(groups[(c, b, h)][0]) for c in range(NCORES))
            T = (mx + 127) // 128
            if T > 0:
                runs.append((TT, h, T))
                TT += T
        blk_runs.append(runs)
    Tmax = max(sum(T for _, _, T in runs) for runs in blk_runs)
    Trun = max(T for runs in blk_runs for _, _, T in runs)

    idx = np.zeros((NCORES, 128, 8 * TT), np.int16)
    dstl = np.full((NCORES, TT, 128), -1, np.int64)
    for b in range(NBLK):
        for (tg, h, T) in blk_runs[b]:
            for c in range(NCORES):
                s, dl = groups[(c, b, h)]
                ne = len(s)
                pad = T * 128 - ne
                sp = np.concatenate([s, np.zeros(pad, np.int64)])
                wrap = sp.reshape(8 * T, 16).T.astype(np.int16)
                idx[c, :, 8 * tg:8 * (tg + T)] = np.tile(wrap, (8, 1))
                dfull = np.concatenate([dl, np.full(pad, -1, np.int64)])
                dstl[c, tg:tg + T] = dfull.reshape(T, 128)
    # one-hots
    oh = np.zeros((NCORES, 128, TT, 128), np.float32)
    oh2 = np.zeros((NCORES, 128, TT, 128), np.float32)
    ci, ti, ei = np.where(dstl >= 0)
    dv = dstl[ci, ti, ei]
    oh[ci, ei, ti, dv] = 1.0
    oh2[ci, dv, ti, ei] = 1.0
    oh8 = oh.reshape(NCORES, 128, TT * 128).astype(NPF8)
    oh28 = oh2.reshape(NCORES, 128, TT * 128).astype(NPF8)
    return idx, oh8, oh28, blk_runs, TT, Tmax, Trun, node_list


def pack_consts(f):
    """Two packed const tensors: CB [128, *] bf16 and CF [128, *] f32."""
    bcols, bparts = {}, []
    fcols, fparts = {}, []

    def addb(name, arr):
        a = np.zeros((128, arr.shape[1]), np.float64)
        a[:arr.shape[0]] = arr
        bcols[name] = (arr.shape[0], sum(p.shape[1] for p in bparts), arr.shape[1])
        bparts.append(a)

    def addf(name, arr):
        a = np.zeros((128, arr.shape[1]), np.float64)
        a[:arr.shape[0]] = arr
        fcols[name] = (arr.shape[0], sum(p.shape[1] for p in fparts), arr.shape[1])
        fparts.append(a)

    addb("W_in", f["W_in"])                      # [128, 64]
    addb("Wl0f", f["Wl0f"]); addb("Wr0f", f["Wr0f"])          # [64, 256]
    addb("Wl1f0", f["Wl1f"][:128]); addb("Wl1f1", f["Wl1f"][128:])
    addb("Wr1f0", f["Wr1f"][:128]); addb("Wr1f1", f["Wr1f"][128:])
    for nm in ("Wl2f_h2", "Wl2f_h1", "Wr2f_h2", "Wr2f_h1"):
        addb(nm + "k0", f[nm][:128]); addb(nm + "k1", f[nm][128:])

    for l in (0, 1):
        addb(f"aT{l}", np.broadcast_to(f[f"aT{l}"][None, :], (128, HC)))
        addb(f"bT{l}", np.broadcast_to(f[f"bT{l}"][None, :], (128, HC)))
    addb("ident", np.eye(128))
    addf("b_in", f["b_in"].reshape(-1, 1))       # [64, 1]

    CB = np.concatenate(bparts, axis=1).astype(NPBF)
    CF = np.concatenate(fparts, axis=1).astype(np.float32)
    return CB, bcols, CF, fcols


# ---------------------------------------------------------------- device
def build(blk_runs, TT, Tmax, Trun, CBw, CFw, BCOLS, FCOLS, K0, K1, K2,
          RHO2, INV_LAM2, DB2):
    nc = bacc.Bacc("TRN2", target_bir_lowering=False, debug=False,
                   num_swdge_queues=4)

    xT = nc.dram_tensor("xT", [D_IN, NPC], BF16, kind="ExternalInput")
    idx = nc.dram_tensor("idx", [128, 8 * TT], I16, kind="ExternalInput")
    oh8 = nc.dram_tensor("oh8", [128, TT * 128], F8, kind="ExternalInput")
    oh28 = nc.dram_tensor("oh28", [128, TT * 128], F8, kind="ExternalInput")
    CB = nc.dram_tensor("CB", [128, CBw], BF16, kind="ExternalInput")
    CF = nc.dram_tensor("CF", [128, CFw], F32, kind="ExternalInput")
    out = nc.dram_tensor("out", [NPC, OUT], F32, kind="ExternalOutput")

    xl0_own = nc.dram_tensor("xl0_own", [NPC, HC], BF16)
    xl0_full = nc.dram_tensor("xl0_full", [N, HC], BF16, addr_space="Shared")
    h1T = nc.dram_tensor("h1T", [HC, NPC], BF16)
    xl1_own = nc.dram_tensor("xl1_own", [NPC, HC], BF16)
    xl1_full = nc.dram_tensor("xl1_full", [N, HC], BF16, addr_space="Shared")
    h2T = nc.dram_tensor("h2T", [HC, NPC], BF16)
    xl2_own = nc.dram_tensor("xl2_own", [NPC, 128], BF16)
    xl2_full = nc.dram_tensor("xl2_full", [N, 128], BF16, addr_space="Shared")

    rg = [list(range(NCORES))]
    qn = [0]

    def next_q():
        qn[0] = (qn[0] + 1) % 4
        return qn[0]

    with tile.TileContext(nc) as tc:
        import contextlib
        with contextlib.ExitStack() as ctx:
            cst = ctx.enter_context(tc.tile_pool(name="cst", bufs=1))
            sb = ctx.enter_context(tc.tile_pool(name="sb", bufs=3))
            gat = ctx.enter_context(tc.tile_pool(name="gat", bufs=6))
            scr = ctx.enter_context(tc.tile_pool(name="scr", bufs=2))
            ps_pxr = ctx.enter_context(tc.tile_pool(name="pspxr", bufs=2, space="PSUM"))
            ps_acc = ctx.enter_context(tc.tile_pool(name="psacc", bufs=2, space="PSUM"))
            ps_misc = ctx.enter_context(tc.tile_pool(name="psmisc", bufs=2, space="PSUM"))

            CBt = cst.tile([128, CBw], BF16)
            nc.sync.dma_start(CBt[:], CB[:])
            CFt = cst.tile([128, CFw], F32)
            nc.sync.dma_start(CFt[:], CF[:])
            idx_t = cst.tile([128, 8 * TT], I16)
            nc.sync.dma_start(idx_t[:], idx[:])
            xrA = cst.tile([128, NBLK * HC], BF16)    # L0 xr'' table
            xrB = cst.tile([128, NBLK * HC], BF16)    # L1 xr'' table
            xr2 = cst.tile([128, NBLK * OUT], BF16)   # L2 xr'' table

            def cb(name):
                r, c0, w = BCOLS[name]
                return CBt[0:r, c0:c0 + w]

            def cf(name):
                r, c0, w = FCOLS[name]
                return CFt[0:r, c0:c0 + w]

            chunks = [(i * 128, min(128, NPC - i * 128)) for i in range(NBLK)]

            # ================= phase A: L0 node tables =================
            for ci, (st, sz) in enumerate(chunks):
                xTc = sb.tile([D_IN, 128], BF16, tag="xTc")
                nc.sync.dma_start(xTc[:, :sz], xT[:, st:st + sz])
                p0 = ps_misc.tile([64, 128], F32, tag="misc")
                nc.tensor.matmul(p0[:, :sz], lhsT=cb("W_in"), rhs=xTc[:, :sz],
                                 start=True, stop=True)
                h0T = sb.tile([64, 128], BF16, tag="h0T")
                nc.scalar.activation(h0T[:, :sz], p0[:, :sz], AF.Relu,
                                     bias=cf("b_in"))
                for Wn, isl in (("Wl0f", False), ("Wr0f", True)):
                    p1 = ps_misc.tile([128, HC], F32, tag="misc")
                    nc.tensor.matmul(p1[:sz, :], lhsT=h0T[:, :sz], rhs=cb(Wn),
                                     start=True, stop=True)
                    if isl:
                        dstap = xrA[:, ci * HC:(ci + 1) * HC]
                        if sz < 128:
                            nc.vector.memset(dstap, 0.0)
                        nc.vector.tensor_copy(dstap[:sz, :], p1[:sz, :])
                    else:
                        cp = sb.tile([128, HC], BF16, tag="cp")
                        nc.vector.tensor_copy(cp[:sz, :], p1[:sz, :])
                        nc.sync.dma_start(xl0_own[st:st + sz, :], cp[:sz, :])

            nc.gpsimd.collective_compute(
                "AllGather", ALU.bypass, ins=[xl0_own[:]], outs=[xl0_full[:]],
                replica_groups=rg)

            # ================= shared edge pass (layers 0/1) =================
            def edge_pass(xl_full, xr_res, Ks, aTn, bTn, hT_dst):
                for b in range(NBLK):
                    runs = blk_runs[b]
                    T_all = sum(T for _, _, T in runs)
                    tg0 = runs[0][0]
                    st = b * BLOCK
                    nreal = min(BLOCK, NPC - st)
                    xr_ap = xr_res[:, b * HC:(b + 1) * HC]

                    ohb = sb.tile([128, Tmax * 128], F8, tag="ohb")
                    nc.sync.dma_start(ohb[:, :T_all * 128],
                                      oh8[:, tg0 * 128:(tg0 + T_all) * 128])
                    oh2b = sb.tile([128, Tmax * 128], F8, tag="oh2b")
                    nc.sync.dma_start(oh2b[:, :T_all * 128],
                                      oh28[:, tg0 * 128:(tg0 + T_all) * 128])

                    Pscr = scr.tile([128, 257 + Tmax * HC], F32, tag="scr")
                    nc.vector.memset(Pscr[:, 0:1], 0.0)

                    gts = []
                    for (tg, hlf, T) in runs:
                        g = gat.tile([128, Trun, HC], BF16, tag="g")
                        src_ap = xl_full[0:HALF, :] if hlf == 0 else xl_full[HALF:N, :]
                        k = 0
                        while k < T:
                            Tc = min(GMAX, T - k)
                            nc.gpsimd.dma_gather(
                                out_ap=g[:, k:k + Tc, :], in_ap=src_ap,
                                idxs_ap=idx_t[:, 8 * (tg + k):8 * (tg + k + Tc)],
                                num_idxs=128 * Tc, num_idxs_reg=128 * Tc,
                                elem_size=HC, queue_num=next_q())
                            k += Tc
                        gts.append((g, T))

                    # scans (leaky prefix) per group
                    tb = 0
                    for (g, T) in gts:
                        for gk in range(0, T, G):
                            Gc = min(G, T - gk)
                            pxr = ps_pxr.tile([128, G, HC], F32, tag="pxr")
                            for t in range(Gc):
                                tbt = tb + gk + t
                                nc.tensor.matmul(
                                    pxr[:, t, :],
                                    lhsT=oh2b[:, tbt * 128:(tbt + 1) * 128],
                                    rhs=xr_ap, start=True, stop=True)
                            o_ap = Pscr[:, 1 + (tb + gk) * HC:1 + (tb + gk + Gc) * HC]
                            nc.vector._custom_dve(
                                SCAN_OP,
                                out=o_ap.rearrange("p (t c) -> p t c", c=HC),
                                in0=g[:, gk:gk + Gc, :],
                                in1=pxr[:, 0:Gc, :], s0=NEG)
                        tb += T

                    # logits via boundary extraction
                    lgf = sb.tile([128, Tmax, HEADS], F32, tag="lgf")
                    for h in range(HEADS):
                        c1 = 64 * h
                        c2 = 64 * h + int(Ks[h])
                        ap2 = Pscr[:, c2:c2 + T_all * HC].rearrange(
                            "p (t o) -> p t o", o=HC)[:, :, 0]
                        ap1 = Pscr[:, c1:c1 + T_all * HC].rearrange(
                            "p (t o) -> p t o", o=HC)[:, :, 0]
                        nc.vector.scalar_tensor_tensor(
                            out=lgf[:, 0:T_all, h], in0=ap2, scalar=2.0,
                            in1=ap1, op0=ALU.mult, op1=ALU.subtract)
                    ap3 = Pscr[:, 64:64 + T_all * HC].rearrange(
                        "p (t h c) -> p t h c", h=HEADS, c=64)[:, :, :, 0]
                    nc.vector.tensor_tensor(out=lgf[:, 0:T_all, :],
                                            in0=lgf[:, 0:T_all, :], in1=ap3,
                                            op=ALU.subtract)

                    # weighted rhs (cols 0:256 = g*ex, 256:260 = ex) + scatter
                    acc = ps_acc.tile([128, 260], F32, tag="acc")
                    tb = 0
                    for (g, T) in gts:
                        for gk in range(0, T, G):
                            Gc = min(G, T - gk)
                            rhs = sb.tile([128, G, 260], BF16, tag="rhs")
                            nc.scalar.activation(rhs[:, 0:Gc, HC:HC + HEADS],
                                                 lgf[:, tb + gk:tb + gk + Gc, :],
                                                 AF.Exp)
                            nc.vector.tensor_tensor(
                                out=rhs[:, 0:Gc, 0:HC].rearrange(
                                    "p t (h c) -> p t h c", h=HEADS),
                                in0=g[:, gk:gk + Gc, :].rearrange(
                                    "p t (h c) -> p t h c", h=HEADS),
                                in1=rhs[:, 0:Gc, HC:HC + HEADS][:, :, :, None]
                                .to_broadcast([128, Gc, HEADS, 64]),
                                op=ALU.mult)
                            for t in range(Gc):
                                tbt = tb + gk + t
                                nc.tensor.matmul(
                                    acc[:], lhsT=ohb[:, tbt * 128:(tbt + 1) * 128],
                                    rhs=rhs[:, t, :],
                                    start=(tbt == 0), stop=(tbt == T_all - 1))
                        tb += T

                    # block post: normalize + BN + relu + transpose out
                    rc4 = sb.tile([128, HEADS], F32, tag="rc4")
                    nc.vector.reciprocal_approx_fast(rc4[:], acc[:, HC:HC + HEADS])
                    m1 = sb.tile([128, HC], BF16, tag="m1")
                    nc.vector.tensor_tensor(
                        out=m1[:].rearrange("p (h c) -> p h c", h=HEADS),
                        in0=acc[:, 0:HC].rearrange("p (h c) -> p h c", h=HEADS),
                        in1=rc4[:, :, None].to_broadcast([128, HEADS, 64]),
                        op=ALU.mult)
                    m2 = sb.tile([128, HC], BF16, tag="m2")
                    nc.vector.tensor_tensor(out=m2[:], in0=m1[:], in1=cb(aTn),
                                            op=ALU.mult)
                    nc.vector.tensor_tensor(out=m2[:], in0=m2[:], in1=cb(bTn),
                                            op=ALU.add)
                    h_ = sb.tile([128, HC], BF16, tag="h_")
                    nc.vector.tensor_scalar(out=h_[:], in0=m2[:], scalar1=0.0,
                                            scalar2=None, op0=ALU.max)
                    for kk in range(2):
                        tp = ps_misc.tile([128, 128], BF16, tag="misc")
                        nc.tensor.transpose(tp[:], h_[:, kk * 128:(kk + 1) * 128],
                                            cb("ident"))
                        hk = sb.tile([128, 128], BF16, tag="hk")
                        nc.scalar.copy(hk[:], tp[:])
                        nc.sync.dma_start(
                            hT_dst[kk * 128:(kk + 1) * 128, st:st + nreal],
                            hk[:, :nreal])

            edge_pass(xl0_full, xrA, K0, "aT0", "bT0", h1T)

            # ================= phase C: L1 node tables =================
            for ci, (st, sz) in enumerate(chunks):
                ht0 = sb.tile([128, 128], BF16, tag="ht0")
                nc.sync.dma_start(ht0[:, :sz], h1T[0:128, st:st + sz])
                ht1 = sb.tile([128, 128], BF16, tag="ht1")
                nc.sync.dma_start(ht1[:, :sz], h1T[128:256, st:st + sz])
                for (W0n, W1n, isl) in (("Wl1f0", "Wl1f1", False),
                                        ("Wr1f0", "Wr1f1", True)):
                    p1 = ps_misc.tile([128, HC], F32, tag="misc")
                    nc.tensor.matmul(p1[:sz, :], lhsT=ht0[:, :sz], rhs=cb(W0n),
                                     start=True, stop=False)
                    nc.tensor.matmul(p1[:sz, :], lhsT=ht1[:, :sz], rhs=cb(W1n),
                                     start=False, stop=True)
                    if isl:
                        dstap = xrB[:, ci * HC:(ci + 1) * HC]
                        if sz < 128:
                            nc.vector.memset(dstap, 0.0)
                        nc.vector.tensor_copy(dstap[:sz, :], p1[:sz, :])
                    else:
                        cp = sb.tile([128, HC], BF16, tag="cp")
                        nc.vector.tensor_copy(cp[:sz, :], p1[:sz, :])
                        nc.sync.dma_start(xl1_own[st:st + sz, :], cp[:sz, :])

            nc.gpsimd.collective_compute(
                "AllGather", ALU.bypass, ins=[xl1_own[:]], outs=[xl1_full[:]],
                replica_groups=rg)

            edge_pass(xl1_full, xrB, K1, "aT1", "bT1", h2T)

            # ================= phase E: L2 node tables =================
            for ci, (st, sz) in enumerate(chunks):
                h2t0 = sb.tile([128, 128], BF16, tag="ht0")
                nc.sync.dma_start(h2t0[:, :sz], h2T[0:128, st:st + sz])
                h2t1 = sb.tile([128, 128], BF16, tag="ht1")
                nc.sync.dma_start(h2t1[:, :sz], h2T[128:256, st:st + sz])
                h1t0 = sb.tile([128, 128], BF16, tag="h1t0")
                nc.sync.dma_start(h1t0[:, :sz], h1T[0:128, st:st + sz])
                h1t1 = sb.tile([128, 128], BF16, tag="h1t1")
                nc.sync.dma_start(h1t1[:, :sz], h1T[128:256, st:st + sz])
                for base, isl in (("Wl2f", False), ("Wr2f", True)):
                    p1 = ps_misc.tile([128, OUT], F32, tag="misc")
                    nc.tensor.matmul(p1[:sz, :], lhsT=h2t0[:, :sz],
                                     rhs=cb(base + "_h2k0"), start=True, stop=False)
                    nc.tensor.matmul(p1[:sz, :], lhsT=h2t1[:, :sz],
                                     rhs=cb(base + "_h2k1"), start=False, stop=False)
                    nc.tensor.matmul(p1[:sz, :], lhsT=h1t0[:, :sz],
                                     rhs=cb(base + "_h1k0"), start=False, stop=False)
                    nc.tensor.matmul(p1[:sz, :], lhsT=h1t1[:, :sz],
                                     rhs=cb(base + "_h1k1"), start=False, stop=True)
                    if isl:
                        dstap = xr2[:, ci * OUT:(ci + 1) * OUT]
                        if sz < 128:
                            nc.vector.memset(dstap, 0.0)
                        nc.vector.tensor_copy(dstap[:sz, :], p1[:sz, :])
                    else:
                        cp2 = sb.tile([128, 128], BF16, tag="cp2")
                        nc.vector.memset(cp2[:], 0.0)
                        nc.vector.tensor_copy(cp2[:sz, 0:OUT], p1[:sz, :])
                        nc.sync.dma_start(xl2_own[st:st + sz, :], cp2[:sz, :])

            nc.gpsimd.collective_compute(
                "AllGather", ALU.bypass, ins=[xl2_own[:]], outs=[xl2_full[:]],
                replica_groups=rg)

            # ================= phase F: L2 edge + log_softmax =================
            dlt_all = cst.tile([128, NBLK], F32)
            for b in range(NBLK):
                runs = blk_runs[b]
                T_all = sum(T for _, _, T in runs)
                tg0 = runs[0][0]
                st = b * BLOCK
                nreal = min(BLOCK, NPC - st)
                xr_ap = xr2[:, b * OUT:(b + 1) * OUT]

                ohb = sb.tile([128, Tmax * 128], F8, tag="ohb")
                nc.sync.dma_start(ohb[:, :T_all * 128],
                                  oh8[:, tg0 * 128:(tg0 + T_all) * 128])
                oh2b = sb.tile([128, Tmax * 128], F8, tag="oh2b")
                nc.sync.dma_start(oh2b[:, :T_all * 128],
                                  oh28[:, tg0 * 128:(tg0 + T_all) * 128])

                Pscr = scr.tile([128, 4 + Tmax * OUT], F32, tag="scr2")
                nc.vector.memset(Pscr[:, 0:1], 0.0)

                gts = []
                for (tg, hlf, T) in runs:
                    g = gat.tile([128, Trun, 128], BF16, tag="g2")
                    src_ap = xl2_full[0:HALF, :] if hlf == 0 else xl2_full[HALF:N, :]
                    k = 0
                    while k < T:
                        Tc = min(GMAX, T - k)
                        nc.gpsimd.dma_gather(
                            out_ap=g[:, k:k + Tc, :], in_ap=src_ap,
                            idxs_ap=idx_t[:, 8 * (tg + k):8 * (tg + k + Tc)],
                            num_idxs=128 * Tc, num_idxs_reg=128 * Tc,
                            elem_size=128, queue_num=next_q())
                        k += Tc
                    gts.append((g, T))

                tb = 0
                for (g, T) in gts:
                    for gk in range(0, T, G):
                        Gc = min(G, T - gk)
                        pxr = ps_pxr.tile([128, G, OUT], F32, tag="pxr")
                        for t in range(Gc):
                            tbt = tb + gk + t
                            nc.tensor.matmul(
                                pxr[:, t, :],
                                lhsT=oh2b[:, tbt * 128:(tbt + 1) * 128],
                                rhs=xr_ap, start=True, stop=True)
                        o_ap = Pscr[:, 1 + (tb + gk) * OUT:1 + (tb + gk + Gc) * OUT]
                        nc.vector._custom_dve(
                            SCAN_OP,
                            out=o_ap.rearrange("p (t c) -> p t c", c=OUT),
                            in0=g[:, gk:gk + Gc, 0:OUT],
                            in1=pxr[:, 0:Gc, :], s0=NEG)
                    tb += T

                lg2 = sb.tile([128, Tmax], F32, tag="lg2")
                ap2 = Pscr[:, K2:K2 + T_all * OUT].rearrange(
                    "p (t o) -> p t o", o=OUT)[:, :, 0]
                ap1 = Pscr[:, 0:T_all * OUT].rearrange(
                    "p (t o) -> p t o", o=OUT)[:, :, 0]
                nc.vector.scalar_tensor_tensor(
                    out=lg2[:, 0:T_all], in0=ap2, scalar=2.0, in1=ap1,
                    op0=ALU.mult, op1=ALU.subtract)
                ap3 = Pscr[:, OUT:OUT + T_all * OUT].rearrange(
                    "p (t o) -> p t o", o=OUT)[:, :, 0]
                nc.vector.tensor_tensor(out=lg2[:, 0:T_all], in0=lg2[:, 0:T_all],
                                        in1=ap3, op=ALU.subtract)
                rhs2 = sb.tile([128, Tmax, 3], BF16, tag="rhs2")
                nc.scalar.activation(rhs2[:, 0:T_all, 2], lg2[:, 0:T_all], AF.Exp)
                tb = 0
                for (g, T) in gts:
                    nc.vector.tensor_tensor(
                        out=rhs2[:, tb:tb + T, 0:OUT], in0=g[:, 0:T, 0:OUT],
                        in1=rhs2[:, tb:tb + T, 2:3].to_broadcast([128, T, OUT]),
                        op=ALU.mult)
                    tb += T
                acc2 = ps_acc.tile([128, 3], F32, tag="acc")
                for tbt in range(T_all):
                    nc.tensor.matmul(acc2[:],
                                     lhsT=ohb[:, tbt * 128:(tbt + 1) * 128],
                                     rhs=rhs2[:, tbt, :],
                                     start=(tbt == 0), stop=(tbt == T_all - 1))
                rc2 = sb.tile([128, 1], F32, tag="rc2")
                nc.vector.reciprocal_approx_fast(rc2[:], acc2[:, 2:3])
                v0 = sb.tile([128, 1], F32, tag="v0")
                nc.vector.tensor_scalar(out=v0[:], in0=acc2[:, RHO2[0]:RHO2[0] + 1],
                                        scalar1=float(INV_LAM2[0]), scalar2=None,
                                        op0=ALU.mult)
                d0 = sb.tile([128, 1], F32, tag="d0")
                nc.vector.scalar_tensor_tensor(
                    out=d0[:], in0=acc2[:, RHO2[1]:RHO2[1] + 1],
                    scalar=float(INV_LAM2[1]), in1=v0[:],
                    op0=ALU.mult, op1=ALU.subtract)
                nc.vector.tensor_scalar(out=dlt_all[:, b:b + 1], in0=d0[:],
                                        scalar1=rc2[:], scalar2=DB2,
                                        op0=ALU.mult, op1=ALU.add)

            # batched softplus tail: one exp + one ln for all blocks
            eall = cst.tile([128, NBLK], F32)
            nc.scalar.activation(eall[:], dlt_all[:], AF.Exp)
            nc.vector.tensor_scalar(out=eall[:], in0=eall[:], scalar1=1.0,
                                    scalar2=None, op0=ALU.add)
            lall = cst.tile([128, NBLK], F32)
            nc.scalar.activation(lall[:], eall[:], AF.Ln)
            for b in range(NBLK):
                st = b * BLOCK
                nreal = min(BLOCK, NPC - st)
                ls = sb.tile([128, 2], F32, tag="ls")
                nc.vector.tensor_scalar(out=ls[:, 0:1], in0=lall[:, b:b + 1],
                                        scalar1=-1.0, scalar2=None, op0=ALU.mult)
                nc.vector.tensor_tensor(out=ls[:, 1:2], in0=dlt_all[:, b:b + 1],
                                        in1=lall[:, b:b + 1], op=ALU.subtract)
                nc.sync.dma_start(out[st:st + nreal, :], ls[:nreal, :])

    nc.compile()
    return nc


# ---------------------------------------------------------------- entry
_CACHE = {}
_PREP_CACHE = {}
LAST_RESULTS = None


def kernel(**inputs):
    global LAST_RESULTS
    import hashlib
    x = np.asarray(inputs["x"], np.float32)
    ei = np.asarray(inputs["edge_index"]).astype(np.int64)

    f = prep_weights(inputs)
    dig = hashlib.blake2b(ei.tobytes(), digest_size=16).hexdigest()
    if dig not in _PREP_CACHE:
        _PREP_CACHE[dig] = preprocess(ei)
    idx, oh8, oh28, blk_runs, TT, Tmax, Trun, node_list = _PREP_CACHE[dig]
    CBa, BCOLS, CFa, FCOLS = pack_consts(f)

    key = (TT, Tmax, tuple(tuple(r) for rs in blk_runs for r in rs),
           tuple(f["k0"]), tuple(f["k1"]), f["k2"], CBa.shape[1], CFa.shape[1],
           tuple(f["rho2"]))
    if key not in _CACHE:
        _CACHE[key] = build(blk_runs, TT, Tmax, Trun, CBa.shape[1], CFa.shape[1],
                            BCOLS, FCOLS, f["k0"], f["k1"], f["k2"],
                            f["rho2"], f["inv_lam2"], f["db2"])
    nc = _CACHE[key]

    in_maps = []
    for c in range(NCORES):
        in_maps.append(dict(
            xT=np.ascontiguousarray(x[node_list[c]].T).astype(NPBF),
            idx=idx[c], oh8=oh8[c], oh28=oh28[c],
            CB=CBa, CF=CFa,
        ))
    res = run_bass_kernel_spmd(nc, in_maps, list(range(NCORES)))
    LAST_RESULTS = res
    full = np.empty((N, OUT), np.float32)
    for c in range(NCORES):
        full[node_list[c]] = res.results[c]["out"].astype(np.float32)
    return full
